# revision 1
# baseline (speedup 1.0000x reference)
"""Block-sparse multi-head attention on 8 Trainium2 NeuronCores.

Problem: y = proj(softmax(mask(q @ k^T / sqrt(hd))) @ v) for
B=2, S=2048, D=1024, H=16 heads, block size 128, with a [16,16] boolean
block mask (True = masked) applied to strictly-upper (k-block > q-block)
blocks.

Sharding: batch x head-group. Core c handles batch c//4 and heads
[4*(c%4), 4*(c%4)+4). No collectives: the host pre-slices inputs
(including pre-transposing x to x^T so the device never transposes) and
sums the 4 per-batch partial projection outputs on the way out.

Device dataflow (per core, matmuls in float32r = full PE rate, ~1e-4 rel):
  qk^T [512,2048]  = w_qk^T @ x^T        (q tiles [q0;q1],[q2;q3]; k tiles
                                          zero-padded per head to K=128 --
                                          K=64 matmuls measure ~2x slower)
  v    [2048,260]  = x^T.T @ w_v_ext     (65 cols/head: 64 v cols + a ones col)
  per head, per k-block ik (mask-specialized at trace time, with the PV
  matmuls software-pipelined one ik behind the QK/exp stage so the PE never
  stalls on ScalarE's exp):
    S^T  [128,q]   = kpad_ik @ q^T       (only visible q runs)
    P~^T [128,q]   = exp(S^T / 8)        (ScalarE; no max subtraction needed:
                                          scores ~ N(0,1), exp is safe in fp32)
    acc  [65,q]   += v_ik_ext^T @ P~^T   (PSUM; row 64 = softmax denominator
                                          via the ones column)
  normalize: acc[0:64] * (1/denom). The denom row is reshaped to [128,16] by
  an SBUF->SBUF DMA (a [1,2048] single-lane reciprocal costs ~13us; the
  reshaped one ~0.2us), reciprocal'd, reshaped back, broadcast across
  partitions on the otherwise-idle GpSimd engine, and multiplied in.
  out  [2048,1024] partial = attn^T.T @ w_proj_slice  (K=128 head pairs,
  odd heads repacked to partitions 64..127 by an SBUF->SBUF DMA)
"""

import numpy as np

import concourse.mybir as mybir
import concourse.tile as tile
from concourse import bacc
from concourse.bass_utils import run_bass_kernel_spmd

B, S, D, H = 2, 2048, 1024, 16
HD = 64          # head dim
BS = 128         # mask block size
NB = S // BS     # 16 blocks per axis
HPC = 4          # heads per core
N_CORES = 8
SCALE = HD ** -0.5

F32 = mybir.dt.float32
F32R = mybir.dt.float32r
EXP = mybir.ActivationFunctionType.Exp

_program_cache: dict[bytes, object] = {}


def _plan_runs(vis, last_vis, ik):
    """Contiguous visible q-block runs for k-block ik, each within one
    512-col PSUM bank window. The PSUM stop flag is sim-only bookkeeping
    (nothing on HW); runs are merged across stop boundaries and the PV
    matmuls pass skip_group_check."""
    runs = []
    jq = 0
    while jq < NB:
        if not vis[jq][ik]:
            jq += 1
            continue
        start = jq
        while jq + 1 < NB and vis[jq + 1][ik] and (jq + 1) % 4 != 0:
            jq += 1
        stopf = any(last_vis[b] == ik for b in range(start, jq + 1))
        runs.append((start, jq - start + 1, stopf))
        jq += 1
    return runs


def _build_program(mask: np.ndarray):
    """Build + compile the (SPMD, mask-specialized) Bass program."""
    # vis[jq][ik]: may q-block jq attend to k-block ik?
    vis = [
        [ik <= jq or not bool(mask[jq, ik]) for ik in range(NB)] for jq in range(NB)
    ]
    last_vis = [max(ik for ik in range(NB) if vis[jq][ik]) for jq in range(NB)]

    nc = bacc.Bacc("TRN2", target_bir_lowering=False, debug=False,
                   num_devices=N_CORES)
    xT_d = nc.dram_tensor("xT", [D, S], F32R, kind="ExternalInput")
    wqk_d = nc.dram_tensor("wqk", [D, HPC * 2 * HD], F32R, kind="ExternalInput")
    wv_d = nc.dram_tensor("wv", [D, HPC * (HD + 1)], F32R, kind="ExternalInput")
    wpr_d = nc.dram_tensor("wpr", [HPC * HD, D], F32R, kind="ExternalInput")
    out_d = nc.dram_tensor("out", [S, D], F32, kind="ExternalOutput")

    KT = D // 128    # 8 k-tiles over the embedding dim
    MT = S // 128    # 16 seq tiles
    VW = HPC * (HD + 1)  # 260

    with tile.TileContext(nc) as tc:
        with tc.tile_pool(name="persist", bufs=1) as pp:
            wpr_t = [pp.tile([128, D], F32R, tag=f"wpr{k}", name=f"wpr{k}")
                     for k in range(2)]
            # q_t[0]=[qT_h0;qT_h1], q_t[1]=[qT_h2;qT_h3]
            q_t = [pp.tile([128, S], F32R, tag=f"q{j}", name=f"q{j}")
                   for j in range(2)]
            # kpad_t[h]: head h's kT in its own 64 partitions, 0 elsewhere,
            # so QK can contract over K=128 (the zero rows contribute 0
            # against the other head's q rows in q_t[h//2])
            kpad_t = [pp.tile([128, S], F32R, tag=f"kp{h}", name=f"kp{h}")
                      for h in range(HPC)]
            v_t = [pp.tile([128, VW], F32R, tag=f"v{m}", name=f"v{m}")
                   for m in range(MT)]
            # head-pair attn tiles for the K=128 projection
            attn_t = [pp.tile([128, S], F32R, tag=f"attn{i}", name=f"attn{i}")
                      for i in range(2)]
            onec_t = pp.tile([128, 1], F32, tag="onec", name="onec")

            nc.vector.memset(onec_t[:], 1.0)
            zsrc_t = pp.tile([64, S], F32, tag="zsrc", name="zsrc")
            nc.vector.memset(zsrc_t[:], 0.0)
            for h in range(HPC):
                z0, z1 = (64, 128) if h % 2 == 0 else (0, 64)
                # f32 -> f32r copy is a rounding producer (plain memset on an
                # f32r tile fails both the ISA check and the f32r verifier)
                nc.vector.tensor_copy(kpad_t[h][z0:z1, :], zsrc_t[:])

            # ---- load x^T and weight slices; project to qk^T and v ----
            with tc.tile_pool(name="inpool", bufs=1) as ip, \
                 tc.tile_pool(name="psB", bufs=3, space="PSUM") as pbp, \
                 tc.tile_pool(name="psC", bufs=2, space="PSUM") as pcp:
                xT_t = [ip.tile([128, S], F32R, tag=f"xT{k}", name=f"xT{k}")
                        for k in range(KT)]
                wqk_t = [ip.tile([128, HPC * 2 * HD], F32R, tag=f"wqk{k}",
                                 name=f"wqk{k}") for k in range(KT)]
                wv_t = [ip.tile([128, VW], F32R, tag=f"wv{k}", name=f"wv{k}")
                        for k in range(KT)]
                # x^T and w_qk pace stage B's first accumulation chain; wv and
                # w_proj are needed later (C / proj), so they queue behind
                for k in range(KT):
                    nc.sync.dma_start(out=wqk_t[k][:], in_=wqk_d[k * 128:(k + 1) * 128, :])
                    nc.sync.dma_start(out=xT_t[k][:], in_=xT_d[k * 128:(k + 1) * 128, :])
                    nc.sync.dma_start(out=wv_t[k][:], in_=wv_d[k * 128:(k + 1) * 128, :])
                for k in range(2):
                    nc.sync.dma_start(out=wpr_t[k][:], in_=wpr_d[k * 128:(k + 1) * 128, :])

                # qk^T: per column tile j, two half-tiles of [128, 1024].
                # j=0,1: q head pairs; j=2,3: k head pairs (split to kpad).
                # Order q01, k01, q23, k23 so head 0/1 attention unblocks first.
                for ci, (j, half) in enumerate(
                        (j, h) for j in (0, 2, 1, 3) for h in range(2)):
                    k0 = (ci * 3) % KT
                    pb = pbp.tile([128, 1024], F32, tag="pb",
                                  name=f"pb{j}{half}")
                    for ki in range(KT):
                        k = (k0 + ki) % KT
                        lhsT = wqk_t[k][:, j * 128:(j + 1) * 128]
                        for c in range(2):
                            cs = half * 1024 + c * 512
                            nc.tensor.matmul(
                                pb[:, c * 512:(c + 1) * 512], lhsT,
                                xT_t[k][:, cs:cs + 512],
                                start=(ki == 0), stop=(ki == KT - 1))
                    hs = half * 1024
                    if j < 2:
                        dst = q_t[j][:, hs:hs + 1024]
                        if j % 2 == 0:
                            nc.vector.tensor_copy(dst, pb[:])
                        else:
                            nc.scalar.copy(dst, pb[:])
                    else:
                        heads = (0, 1) if j == 2 else (2, 3)
                        nc.vector.tensor_copy(
                            kpad_t[heads[0]][0:64, hs:hs + 1024], pb[0:64, :])
                        nc.scalar.copy(
                            kpad_t[heads[1]][64:128, hs:hs + 1024], pb[64:128, :])

                # v_ext: natural layout [seq, 260]
                for m in range(MT):
                    pc = pcp.tile([128, VW], F32, tag="pc", name=f"pc{m}")
                    k0 = (m * 5) % KT
                    for ki in range(KT):
                        k = (k0 + ki) % KT
                        nc.tensor.matmul(
                            pc[:], xT_t[k][:, m * 128:(m + 1) * 128], wv_t[k][:],
                            start=(ki == 0), stop=(ki == KT - 1))
                    nc.vector.tensor_copy(v_t[m][:], pc[:])
                    for j in range(HPC):
                        oc = j * (HD + 1) + HD
                        nc.vector.tensor_copy(v_t[m][:, oc:oc + 1], onec_t[:])

            # ---- attention (mask-specialized) + normalize ----
            with tc.tile_pool(name="atpool", bufs=1) as ap, \
                 tc.tile_pool(name="psA", bufs=1, space="PSUM") as pap, \
                 tc.tile_pool(name="psS", bufs=2, space="PSUM") as psp:
                lastw = [max(last_vis[jq] for jq in range(w * 4, w * 4 + 4))
                         for w in range(4)]
                _odd = {}

                def norm_window(j, pa, w):
                    """Normalize q-window w of head j as soon as its PSUM
                    accumulation is complete (its last visible k-block)."""
                    ws = w * 512
                    if j % 2 == 0:
                        dst = attn_t[j // 2][0:64, ws:ws + 512]
                    else:
                        dst = _odd[j][0:64, ws:ws + 512]
                    dnr = ap.tile([65, 512], F32, tag="dnr", bufs=3,
                                  name=f"dnr{j}_{w}")
                    nc.vector.tensor_copy(dnr[64:65, :], pa[64:65, ws:ws + 512])
                    nc.vector.tensor_copy(dst, pa[0:64, ws:ws + 512])
                    d16 = ap.tile([128, 4], F32, tag="d16", bufs=3,
                                  name=f"d16_{j}_{w}")
                    nc.gpsimd.dma_start(out=d16[:], in_=dnr[64:65, :])
                    nc.vector.reciprocal(d16[:], d16[:])
                    r0 = ap.tile([1, 512], F32, tag="r0", bufs=3,
                                 name=f"r0_{j}_{w}")
                    nc.gpsimd.dma_start(out=r0[:], in_=d16[:])
                    dbc = ap.tile([64, 512], F32, tag="dbc", bufs=3,
                                  name=f"dbc{j}_{w}")
                    nc.gpsimd.partition_broadcast(dbc[:], r0[:])
                    nc.vector.tensor_mul(dst, dst, dbc[:])
                    if j % 2 == 1:
                        nc.gpsimd.dma_start(
                            out=attn_t[j // 2][64:128, ws:ws + 512],
                            in_=_odd[j][0:64, ws:ws + 512])

                for j in (1, 0, 3, 2):
                    qtile = q_t[j // 2]
                    if j % 2 == 1:
                        _odd[j] = ap.tile([64, S], F32R, tag="oddh", bufs=2,
                                          name=f"oddh{j}")
                    pa = pap.tile([65, S], F32, tag="pa", name=f"pa{j}")
                    # software pipeline: PV trails QK/exp by one k-block so
                    # the PE never waits on ScalarE's exp latency. Runs are
                    # grouped into 1024-col window pairs sharing one exp op
                    # (the ~350-cycle ACT per-op overhead dominates at run
                    # granularity; exp over unwritten gap columns is safe --
                    # stale PSUM holds bounded pre-softmax scores).
                    pending = []
                    for ik in range(NB):
                        lhsT_k = kpad_t[j][:, ik * 128:(ik + 1) * 128]
                        lhsT_v = v_t[ik][:, j * (HD + 1):(j + 1) * (HD + 1)]
                        new_pending = []
                        runs = _plan_runs(vis, last_vis, ik)
                        for g in range(2):
                            gb = g * 1024
                            gruns = [r for r in runs if gb <= r[0] * 128 < gb + 1024]
                            if not gruns:
                                continue
                            lo = min(r[0] * 128 for r in gruns) - gb
                            hi = max((r[0] + r[1]) * 128 for r in gruns) - gb
                            stg = psp.tile([128, 1024], F32, tag="st",
                                           name=f"st{j}_{ik}_{g}")
                            for (qb0, nbk, stopf) in gruns:
                                qs, qlen = qb0 * 128, nbk * 128
                                nc.tensor.matmul(
                                    stg[:, qs - gb:qs - gb + qlen], lhsT_k,
                                    qtile[:, qs:qs + qlen],
                                    start=True, stop=True)
                            ptg = ap.tile([128, 1024], F32R, tag="pt", bufs=4,
                                          name=f"pt{j}_{ik}_{g}")
                            nc.scalar.activation(ptg[:, lo:hi], stg[:, lo:hi],
                                                 EXP, scale=SCALE)
                            for (qb0, nbk, stopf) in gruns:
                                qs, qlen = qb0 * 128, nbk * 128
                                new_pending.append(
                                    (lhsT_v, ptg, gb, qs, qlen, ik == 0, stopf))
                        for (lv, ptg, gb, qs, qlen, startf, stopf) in pending:
                            nc.tensor.matmul(pa[0:65, qs:qs + qlen], lv,
                                             ptg[:, qs - gb:qs - gb + qlen],
                                             start=startf, stop=stopf,
                                             skip_group_check=True)
                        if ik > 0:
                            for w in range(4):
                                if lastw[w] == ik - 1:
                                    norm_window(j, pa, w)
                        pending = new_pending
                    for (lv, ptg, gb, qs, qlen, startf, stopf) in pending:
                        nc.tensor.matmul(pa[0:65, qs:qs + qlen], lv,
                                         ptg[:, qs - gb:qs - gb + qlen],
                                         start=startf, stop=stopf,
                                         skip_group_check=True)
                    for w in range(4):
                        if lastw[w] == NB - 1:
                            norm_window(j, pa, w)

            # ---- output projection (partial; host sums across head groups) ----
            with tc.tile_pool(name="opool", bufs=2) as op, \
                 tc.tile_pool(name="psO", bufs=2, space="PSUM") as pop:
                worder = sorted(range(4), key=lambda w: lastw[w])
                for m in [w * 4 + i for w in worder for i in range(4)]:
                    po = pop.tile([128, D], F32, tag="po", name=f"po{m}")
                    for kt in range(2):
                        lhsT = attn_t[kt][:, m * 128:(m + 1) * 128]
                        for c in range(2):
                            nc.tensor.matmul(
                                po[:, c * 512:(c + 1) * 512], lhsT,
                                wpr_t[kt][:, c * 512:(c + 1) * 512],
                                start=(kt == 0), stop=(kt == 1))
                    ob = op.tile([128, D], F32, tag="ob", name=f"ob{m}")
                    if m % 2 == 0:
                        nc.vector.tensor_copy(ob[:], po[:])
                    else:
                        nc.scalar.copy(ob[:], po[:])
                    nc.sync.dma_start(out=out_d[m * 128:(m + 1) * 128, :],
                                      in_=ob[:])

    nc.compile()
    return nc


def _host_prep(x, w_qkv, w_proj):
    """Per-core input slices (all float32, C-contiguous)."""
    xT = [np.ascontiguousarray(x[b].T) for b in range(B)]
    in_maps = []
    for c in range(N_CORES):
        b, g = c // 4, c % 4
        heads = range(g * HPC, (g + 1) * HPC)
        wqk = np.empty((D, HPC * 2 * HD), np.float32)
        wv = np.zeros((D, HPC * (HD + 1)), np.float32)
        wpr = np.empty((HPC * HD, D), np.float32)
        for j, h in enumerate(heads):
            # layout: [q0 q1 q2 q3 k0 k1 k2 k3], 64 cols each
            wqk[:, j * HD:(j + 1) * HD] = w_qkv[:, h * HD:(h + 1) * HD]
            wqk[:, HPC * HD + j * HD:HPC * HD + (j + 1) * HD] = \
                w_qkv[:, D + h * HD:D + (h + 1) * HD]
            wv[:, j * (HD + 1):j * (HD + 1) + HD] = \
                w_qkv[:, 2 * D + h * HD:2 * D + (h + 1) * HD]
            # w_proj rows ordered to match attn head-pair packing
            wpr[j * HD:(j + 1) * HD, :] = w_proj[h * HD:(h + 1) * HD, :]
        in_maps.append({
            "xT": xT[b],
            "wqk": np.ascontiguousarray(wqk),
            "wv": np.ascontiguousarray(wv),
            "wpr": np.ascontiguousarray(wpr),
        })
    return in_maps


def get_program(block_mask: np.ndarray):
    key = np.asarray(block_mask, bool).tobytes()
    if key not in _program_cache:
        _program_cache[key] = _build_program(np.asarray(block_mask, bool))
    return _program_cache[key]


def kernel(x, w_qkv, w_proj, b_proj, block_mask):
    x = np.asarray(x, np.float32)
    w_qkv = np.asarray(w_qkv, np.float32)
    w_proj = np.asarray(w_proj, np.float32)
    b_proj = np.asarray(b_proj, np.float32)
    nc = get_program(block_mask)
    in_maps = _host_prep(x, w_qkv, w_proj)
    res = run_bass_kernel_spmd(nc, in_maps, core_ids=list(range(N_CORES)))
    out = np.empty((B, S, D), np.float32)
    for b in range(B):
        acc = res.results[4 * b]["out"].astype(np.float64)
        for g in range(1, 4):
            acc = acc + res.results[4 * b + g]["out"]
        out[b] = (acc + b_proj).astype(np.float32)
    return out



# revision 9
# speedup vs baseline: 1.1290x; 1.1290x over previous
"""Block-sparse multi-head attention on 8 Trainium2 NeuronCores.

Problem: y = proj(softmax(mask(q @ k^T / sqrt(hd))) @ v) for
B=2, S=2048, D=1024, H=16 heads, block size 128, with a [16,16] boolean
block mask (True = masked) applied to strictly-upper (k-block > q-block)
blocks.

Sharding: batch x head-group. Core c handles batch c//4 and heads
[4*(c%4), 4*(c%4)+4). No collectives: the host pre-slices inputs
(including pre-transposing x to x^T) and sums the 4 per-batch partial
projection outputs on the way out.

This version fuses all phases into one software-pipelined instruction
stream to keep ScalarE (the exp bottleneck, ~100us/core) and the PE
(~123us/core) simultaneously busy:
  - x/w_qkv/w_v are uploaded in bf16 (halves input DMA to ~6.5MB);
    DMAs are chunked by xT column-slice and issued in consumption order
    so the first attention exp lands ~8us into the kernel.
  - qk-gen for head pair 0 runs first; v-gen and pair-1 qk-gen chunks
    are interleaved into the attention pipeline of heads 1 and 0
    (sharing one PSUM ring) so the PE never idles long enough for HAM
    to re-throttle the clock.
  - attention per head runs as two window passes g=0/1 (pa [65,1024]
    PSUM x2-ring), per k-block: S^T = kpad_ik @ q^T (runs), P~^T =
    exp(S^T/8) (ScalarE, one op per (ik, 1024-window)), PV accumulated
    into pa with the ones-column denominator trick (row 64).
  - normalization: only the two PSUM->SBUF copies are eager; the
    reciprocal/broadcast/multiply chain (which round-trips SBUF DMAs)
    is deferred and spread over the next head's iterations so it never
    head-of-line-blocks the in-order Vector/GpSimd queues.
  - projection is a 4-deep PSUM pipeline (alternating ring slots) with
    PSUM->SBUF copies alternating Vector/Scalar and per-tile output
    DMAs; m-tiles ordered so the last head's deferred normalize chain
    overlaps the first half of proj.
"""

import numpy as np
from ml_dtypes import bfloat16

import concourse.mybir as mybir
import concourse.tile as tile
from concourse import bacc
from concourse.bass_utils import run_bass_kernel_spmd

B, S, D, H = 2, 2048, 1024, 16
HD = 64          # head dim
BS = 128         # mask block size
NB = S // BS     # 16 blocks per axis
HPC = 4          # heads per core
N_CORES = 8
SCALE = HD ** -0.5
KT = D // 128    # 8 k-tiles over the embedding dim
VW = HPC * (HD + 1)  # 260

F32 = mybir.dt.float32
F32R = mybir.dt.float32r
BF16 = mybir.dt.bfloat16
EXP = mybir.ActivationFunctionType.Exp

_program_cache: dict[bytes, object] = {}


def _plan_runs_g(vis, last_vis, ik, g):
    """Contiguous visible q-block runs for k-block ik within 1024-col
    window g. Runs break at 4-block (512-col = PSUM bank) boundaries."""
    runs = []
    jq, end = 8 * g, 8 * g + 8
    while jq < end:
        if not vis[jq][ik]:
            jq += 1
            continue
        start = jq
        while jq + 1 < end and vis[jq + 1][ik] and (jq + 1) % 4 != 0:
            jq += 1
        stopf = any(last_vis[b] == ik for b in range(start, jq + 1))
        runs.append((start, jq - start + 1, stopf))
        jq += 1
    return runs


def _build_program(mask: np.ndarray):
    vis = [[ik <= jq or not bool(mask[jq, ik]) for ik in range(NB)]
           for jq in range(NB)]
    last_vis = [max(ik for ik in range(NB) if vis[jq][ik]) for jq in range(NB)]
    lastw = [max(last_vis[w * 4:(w + 1) * 4]) for w in range(4)]
    RUNS = {(g, ik): _plan_runs_g(vis, last_vis, ik, g)
            for g in range(2) for ik in range(NB)}

    nc = bacc.Bacc("TRN2", target_bir_lowering=False, debug=False,
                   num_devices=N_CORES)
    xT_d = nc.dram_tensor("xT", [D, S], BF16, kind="ExternalInput")
    # host layout: cols [0:128]=[q0|q1] [128:256]=[k0|k1]
    #              [256:384]=[q2|q3] [384:512]=[k2|k3]
    wqk_d = nc.dram_tensor("wqk", [D, 2 * HPC * HD], BF16, kind="ExternalInput")
    wv_d = nc.dram_tensor("wv", [D, VW], BF16, kind="ExternalInput")
    wpr_d = nc.dram_tensor("wpr", [HPC * HD, D], F32R, kind="ExternalInput")
    out_d = nc.dram_tensor("out", [S, D], F32, kind="ExternalOutput")

    with tile.TileContext(nc) as tc:
        with tc.tile_pool(name="pp", bufs=1) as pp, \
             tc.tile_pool(name="ptp", bufs=5) as ptp, \
             tc.tile_pool(name="ps", bufs=2, space="PSUM") as ps:
            # ---- persistent SBUF tiles ----
            xT_t = [pp.tile([128, S], BF16, tag=f"xT{k}", name=f"xT{k}")
                    for k in range(KT)]
            wqk_t = [pp.tile([128, 2 * HPC * HD], BF16, tag=f"wqk{k}",
                             name=f"wqk{k}") for k in range(KT)]
            wv_t = [pp.tile([128, VW], BF16, tag=f"wv{k}", name=f"wv{k}")
                    for k in range(KT)]
            wpr_t = [pp.tile([128, D], F32R, tag=f"wpr{k}", name=f"wpr{k}")
                     for k in range(2)]
            q_t = [pp.tile([128, S], F32R, tag=f"q{p}", name=f"q{p}")
                   for p in range(2)]
            kpad_t = [pp.tile([128, S], F32R, tag=f"kp{h}", name=f"kp{h}")
                      for h in range(HPC)]
            v_t = [pp.tile([128, VW], F32R, tag=f"v{m}", name=f"v{m}")
                   for m in range(NB)]
            attn_t = [pp.tile([128, S], F32R, tag=f"attn{i}", name=f"attn{i}")
                      for i in range(2)]
            d16_t = pp.tile([128, 8 * HPC * 2], F32, tag="d16", name="d16")
            onec = pp.tile([128, 4], F32, tag="onec", name="onec")
            zsrc = pp.tile([64, 512], F32, tag="zsrc", name="zsrc")
            scr = pp.tile([128, 4], F32, tag="scr", name="scr")

            # ---- init + ACT table pre-warm ----
            nc.vector.memset(onec[:], 1.0)
            nc.vector.memset(zsrc[:], 0.0)
            nc.scalar.activation(scr[:], onec[:], EXP, scale=1.0)
            for h in range(HPC):
                z0 = 64 if h % 2 == 0 else 0
                for c in range(4):
                    eng = nc.vector if (h * 4 + c) % 2 == 0 else nc.scalar
                    cs = c * 512
                    if eng is nc.vector:
                        eng.tensor_copy(kpad_t[h][z0:z0 + 64, cs:cs + 512],
                                        zsrc[:])
                    else:
                        eng.copy(kpad_t[h][z0:z0 + 64, cs:cs + 512], zsrc[:])

            # ---- input DMAs in consumption order ----
            # wqk pair0; xT slices 0,1; wv; xT slices 2,3; wqk pair1; wpr
            for k in range(KT):
                nc.sync.dma_start(out=wqk_t[k][:, 0:256],
                                  in_=wqk_d[k * 128:(k + 1) * 128, 0:256])
            for s in (0, 1):
                for k in range(KT):
                    nc.sync.dma_start(
                        out=xT_t[k][:, s * 512:(s + 1) * 512],
                        in_=xT_d[k * 128:(k + 1) * 128, s * 512:(s + 1) * 512])
            for k in range(KT):
                nc.sync.dma_start(out=wv_t[k][:],
                                  in_=wv_d[k * 128:(k + 1) * 128, :])
            for s in (2, 3):
                for k in range(KT):
                    nc.sync.dma_start(
                        out=xT_t[k][:, s * 512:(s + 1) * 512],
                        in_=xT_d[k * 128:(k + 1) * 128, s * 512:(s + 1) * 512])
            for k in range(KT):
                nc.sync.dma_start(out=wqk_t[k][:, 256:512],
                                  in_=wqk_d[k * 128:(k + 1) * 128, 256:512])
            for k in range(2):
                nc.sync.dma_start(out=wpr_t[k][:],
                                  in_=wpr_d[k * 128:(k + 1) * 128, :])

            # ---- gen chunk emitters ----
            def qk_chunk(p, t, c, lead=False):
                """[128,512] chunk of q-pair (t=0) or k-pair (t=1) tile."""
                pb = ps.tile([128, 512], F32, tag="st", name=f"pb{p}{t}{c}")
                off = p * 256 + t * 128
                cs = c * 512
                for k in range(KT):
                    nc.tensor.matmul(pb[:], wqk_t[k][:, off:off + 128],
                                     xT_t[k][:, cs:cs + 512],
                                     start=(k == 0), stop=(k == KT - 1))
                if t == 0:
                    if lead and c % 2 == 1:
                        nc.scalar.copy(q_t[p][:, cs:cs + 512], pb[:])
                    else:
                        nc.vector.tensor_copy(q_t[p][:, cs:cs + 512], pb[:])
                else:
                    h0, h1 = 2 * p, 2 * p + 1
                    if lead and c % 2 == 1:
                        nc.scalar.copy(kpad_t[h0][0:64, cs:cs + 512],
                                       pb[0:64, :])
                        nc.vector.tensor_copy(kpad_t[h1][64:128, cs:cs + 512],
                                              pb[64:128, :])
                    else:
                        nc.vector.tensor_copy(kpad_t[h0][0:64, cs:cs + 512],
                                              pb[0:64, :])
                        nc.vector.tensor_copy(kpad_t[h1][64:128, cs:cs + 512],
                                              pb[64:128, :])

            def v_chunk(m):
                pc = ps.tile([128, 512], F32, tag="st", name=f"pc{m}")
                for k in range(KT):
                    nc.tensor.matmul(pc[:, 0:VW],
                                     xT_t[k][:, m * 128:(m + 1) * 128],
                                     wv_t[k][:],
                                     start=(k == 0), stop=(k == KT - 1))
                nc.vector.tensor_copy(v_t[m][:], pc[:, 0:VW])
                nc.vector.tensor_copy(v_t[m][:, HD::HD + 1], onec[:])

            # ---- deferred-op machinery ----
            deferred = []  # [countdown, fn]

            def poll_deferred():
                due = [d for d in deferred if d[0] <= 1]
                for d in due:
                    deferred.remove(d)
                for d in deferred:
                    d[0] -= 1
                for d in due:
                    d[1]()

            def force_deferred(keep=None):
                kept = []
                while deferred:
                    d = deferred.pop(0)
                    if keep is not None and d[2] == keep:
                        kept.append(d)
                    else:
                        d[1]()
                deferred.extend(kept)

            # ---- normalize chain ----
            first_mm = {}   # (j, w) -> True once consumed
            wins_done = {}  # (j, g) -> count

            # per-head staging, ring-allocated (lifetimes span into next head)
            cur = {"den": None, "odd": None}

            def enqueue_chain(j, g, spacing):
                p, gc = j // 2, g * 1024
                sl = d16_t[:, (2 * j + g) * 8:(2 * j + g + 1) * 8]
                den, odd = cur["den"], cur["odd"]
                if j % 2 == 0:
                    dst = attn_t[p][0:64, gc:gc + 1024]
                else:
                    dst = odd[0:64, gc:gc + 1024]

                def s1():
                    nc.gpsimd.dma_start(out=sl, in_=den[64:65, gc:gc + 1024])

                def s2():
                    nc.vector.reciprocal(sl, sl)

                def s3():
                    nc.gpsimd.dma_start(out=den[0:1, gc:gc + 1024], in_=sl)

                def s4():
                    dbc = pp.tile([64, 1024], F32, tag="dbc", bufs=2,
                                  name=f"dbc{j}{g}")
                    nc.gpsimd.partition_broadcast(dbc[:],
                                                  den[0:1, gc:gc + 1024])
                    nc.vector.tensor_mul(dst, dst, dbc[:])

                def s5():
                    nc.gpsimd.dma_start(out=attn_t[p][64:128, gc:gc + 1024],
                                        in_=odd[0:64, gc:gc + 1024])

                steps = [s1, s2, s3, s4] + ([s5] if j % 2 == 1 else [])
                cd = 1
                for i, fn in enumerate(steps):
                    deferred.append([cd, fn, (j, g)])
                    cd += spacing

            def norm_copies(j, g, ik, pa_g):
                for w in (2 * g, 2 * g + 1):
                    if lastw[w] != ik:
                        continue
                    ws = w * 512
                    rel = ws - g * 1024
                    if j % 2 == 0:
                        dst = attn_t[j // 2][0:64, ws:ws + 512]
                    else:
                        dst = cur["odd"][0:64, ws:ws + 512]
                    nc.vector.tensor_copy(dst, pa_g[0:64, rel:rel + 512])
                    nc.vector.tensor_copy(cur["den"][64:65, ws:ws + 512],
                                          pa_g[64:65, rel:rel + 512])
                    wins_done[(j, g)] = wins_done.get((j, g), 0) + 1
                    if wins_done[(j, g)] == 2:
                        enqueue_chain(j, g, spacing=3)

            # ---- attention pipeline ----
            pending = [None]  # [(j, g, ik, runs, ptg, pa_g)]

            def flush_pending():
                item = pending[0]
                pending[0] = None
                if item is None:
                    return
                j, g, ik, runs, ptg, pa_g = item
                lhsT_v = v_t[ik][:, j * (HD + 1):(j + 1) * (HD + 1)]
                for (qb0, nbk, stopf) in runs:
                    qs, qlen = qb0 * 128, nbk * 128
                    rel = qs - g * 1024
                    w = qb0 // 4
                    startf = first_mm.pop((j, w), False)
                    nc.tensor.matmul(pa_g[0:65, rel:rel + qlen], lhsT_v,
                                     ptg[:, rel:rel + qlen],
                                     start=startf, stop=stopf,
                                     skip_group_check=True)
                norm_copies(j, g, ik, pa_g)

            def attn_iter(j, g, ik, pa_g, gen=None):
                runs = RUNS[(g, ik)]
                stg = ps.tile([128, 1024], F32, tag="st", name=f"st{j}{g}{ik}")
                lhsT_k = kpad_t[j][:, ik * 128:(ik + 1) * 128]
                qtile = q_t[j // 2]
                for (qb0, nbk, stopf) in runs:
                    qs, qlen = qb0 * 128, nbk * 128
                    rel = qs - g * 1024
                    nc.tensor.matmul(stg[:, rel:rel + qlen], lhsT_k,
                                     qtile[:, qs:qs + qlen],
                                     start=True, stop=True)
                lo = min(r[0] for r in runs) * 128 - g * 1024
                hi = (max(r[0] + r[1] for r in runs)) * 128 - g * 1024
                ptg = ptp.tile([128, 1024], F32R, tag="pt", name=f"pt{j}{g}{ik}")
                nc.scalar.activation(ptg[:, lo:hi], stg[:, lo:hi], EXP,
                                     scale=SCALE)
                if gen is not None:
                    gen()
                poll_deferred()
                flush_pending()
                pending[0] = (j, g, ik, runs, ptg, pa_g)

            # ---- lead: qk-gen for pair 0, windows g=0 ----
            for (t, c) in ((0, 0), (1, 0), (0, 1), (1, 1)):
                qk_chunk(0, t, c, lead=True)

            iters = {g: [ik for ik in range(NB) if RUNS[(g, ik)]]
                     for g in range(2)}

            def head_items(j):
                phases = [(0, [ik for ik in iters[0] if ik < 8]),
                          (1, [ik for ik in iters[1] if ik < 8]),
                          (0, [ik for ik in iters[0] if ik >= 8]),
                          (1, [ik for ik in iters[1] if ik >= 8])]
                return [(g, ik) for (g, iklist) in phases for ik in iklist]

            def edf_schedule(items, chunks):
                """Assign gen chunks to iteration slots by earliest deadline.
                chunks: list of (deadline_slot_inclusive, fn). Returns
                slot -> [fns]; infeasible chunks go to slot 0."""
                slots = {i: [] for i in range(len(items))}
                fill = {i: 0 for i in range(len(items))}
                for dl, fn in sorted(chunks, key=lambda c: c[0]):
                    placed = False
                    # latest-fit: emit just-in-time so gen MMs queue behind
                    # already-arrived DMA data instead of stalling the PE
                    for s in range(min(dl, len(items) - 1), -1, -1):
                        if fill[s] < 2:
                            slots[s].append(fn)
                            fill[s] += 1
                            placed = True
                            break
                    if not placed:
                        slots[0].insert(0, fn)
                return slots

            def head1_chunks(items):
                """v tiles (PV deadline) + pair-0 windows g=1 (QK deadline)."""
                chunks = []
                for m in range(NB):
                    idx = min((i for i, (g, ik) in enumerate(items) if ik == m),
                              default=0)
                    chunks.append((idx + 1, lambda m=m: v_chunk(m)))
                for c in (2, 3):
                    # q chunk c: first QK of window-pair g=c//2 touching it
                    idx = min((i for i, (g, ik) in enumerate(items)
                               if g == c // 2), default=1)
                    chunks.append((max(0, idx - 1),
                                   lambda c=c: qk_chunk(0, 0, c)))
                    # kpad chunk c: first QK with ik in [4c, 4c+4)
                    idx = min((i for i, (g, ik) in enumerate(items)
                               if 4 * c <= ik < 4 * c + 4), default=1)
                    chunks.append((max(0, idx - 1),
                                   lambda c=c: qk_chunk(0, 1, c)))
                return chunks

            def head0_chunks(items):
                """pair-1 gen, needed only by heads 3/2: spread evenly."""
                chunks = []
                pos = 0
                for c in range(4):
                    for t in (0, 1):
                        chunks.append((pos, lambda t=t, c=c: qk_chunk(1, t, c)))
                        pos += 3
                return chunks

            for j in (1, 0, 3, 2):
                items = head_items(j)
                if j == 1:
                    genmap = edf_schedule(items, head1_chunks(items))
                elif j == 0:
                    genmap = edf_schedule(items, head0_chunks(items))
                else:
                    genmap = {}
                for w in range(4):
                    first_mm[(j, w)] = True
                cur["den"] = pp.tile([65, S], F32, tag="den", bufs=2,
                                     name=f"den{j}")
                if j % 2 == 1:
                    cur["odd"] = pp.tile([64, S], F32R, tag="odd", bufs=1,
                                         name=f"odd{j}")
                pa = {}
                for i, (g, ik) in enumerate(items):
                    if g not in pa:
                        pa[g] = ps.tile([65, 1024], F32, tag="pa",
                                        name=f"pa{j}{g}")
                    fns = genmap.get(i, [])
                    gen = (lambda fns=fns: [f() for f in fns]) if fns else None
                    attn_iter(j, g, ik, pa[g], gen=gen)
                flush_pending()
                for w in range(4):
                    first_mm.pop((j, w), None)

            # ---- projection + output ----
            # flush all chains except the last head's g=1 (interleaved below)
            force_deferred(keep=(2, 1))
            last_chain = [d for d in deferred if d[2] == (2, 1)]
            deferred.clear()
            for mi, m in enumerate(list(range(8)) + list(range(8, 16))):
                if last_chain and mi % 2 == 0:
                    last_chain.pop(0)[1]()
                if m == 8:
                    while last_chain:
                        last_chain.pop(0)[1]()
                po = ps.tile([128, D], F32, tag=("st" if mi % 2 == 0 else "pa"),
                             name=f"po{m}")
                for kt in range(2):
                    for c in range(2):
                        nc.tensor.matmul(
                            po[:, c * 512:(c + 1) * 512],
                            attn_t[kt][:, m * 128:(m + 1) * 128],
                            wpr_t[kt][:, c * 512:(c + 1) * 512],
                            start=(kt == 0), stop=(kt == 1))
                ob = pp.tile([128, D], F32, tag="ob", bufs=3, name=f"ob{m}")
                if mi % 2 == 0:
                    nc.vector.tensor_copy(ob[:], po[:])
                else:
                    nc.scalar.copy(ob[:], po[:])
                nc.sync.dma_start(out=out_d[m * 128:(m + 1) * 128, :],
                                  in_=ob[:])
            while last_chain:
                last_chain.pop(0)[1]()

    # consume first_mm flags at first-visible ik
    nc.compile()
    return nc


def _host_prep(x, w_qkv, w_proj):
    """Per-core input slices. x/wqk/wv in bf16, wpr in f32."""
    xT = [np.ascontiguousarray(x[b].T).astype(bfloat16) for b in range(B)]
    in_maps = []
    for c in range(N_CORES):
        b, grp = c // 4, c % 4
        heads = list(range(grp * HPC, (grp + 1) * HPC))
        wqk = np.empty((D, 2 * HPC * HD), np.float32)
        wv = np.zeros((D, VW), np.float32)
        wpr = np.empty((HPC * HD, D), np.float32)
        for j, h in enumerate(heads):
            p, i = j // 2, j % 2  # pair, index in pair
            # pair block: [q_a|q_b][k_a|k_b] at 256*p
            wqk[:, p * 256 + i * HD:p * 256 + (i + 1) * HD] = \
                w_qkv[:, h * HD:(h + 1) * HD]
            wqk[:, p * 256 + 128 + i * HD:p * 256 + 128 + (i + 1) * HD] = \
                w_qkv[:, D + h * HD:D + (h + 1) * HD]
            wv[:, j * (HD + 1):j * (HD + 1) + HD] = \
                w_qkv[:, 2 * D + h * HD:2 * D + (h + 1) * HD]
            wpr[j * HD:(j + 1) * HD, :] = w_proj[h * HD:(h + 1) * HD, :]
        in_maps.append({
            "xT": xT[b],
            "wqk": np.ascontiguousarray(wqk).astype(bfloat16),
            "wv": np.ascontiguousarray(wv).astype(bfloat16),
            "wpr": np.ascontiguousarray(wpr),
        })
    return in_maps


def get_program(block_mask: np.ndarray):
    key = np.asarray(block_mask, bool).tobytes()
    if key not in _program_cache:
        _program_cache[key] = _build_program(np.asarray(block_mask, bool))
    return _program_cache[key]


def kernel(x, w_qkv, w_proj, b_proj, block_mask):
    x = np.asarray(x, np.float32)
    w_qkv = np.asarray(w_qkv, np.float32)
    w_proj = np.asarray(w_proj, np.float32)
    b_proj = np.asarray(b_proj, np.float32)
    nc = get_program(block_mask)
    in_maps = _host_prep(x, w_qkv, w_proj)
    res = run_bass_kernel_spmd(nc, in_maps, core_ids=list(range(N_CORES)))
    out = np.empty((B, S, D), np.float32)
    for b in range(B):
        acc = res.results[4 * b]["out"].astype(np.float64)
        for g in range(1, 4):
            acc = acc + res.results[4 * b + g]["out"]
        out[b] = (acc + b_proj).astype(np.float32)
    return out


# revision 17
# speedup vs baseline: 1.1569x; 1.0248x over previous
"""Block-sparse multi-head attention on 8 Trainium2 NeuronCores.

Problem: y = proj(softmax(mask(q @ k^T / sqrt(hd))) @ v) for
B=2, S=2048, D=1024, H=16 heads, block size 128, with a [16,16] boolean
block mask (True = masked) applied to strictly-upper (k-block > q-block)
blocks.

Sharding: batch x head-group. Core c handles batch c//4 and heads
[4*(c%4), 4*(c%4)+4). No collectives: the host pre-slices inputs
(including pre-transposing x to x^T) and sums the 4 per-batch partial
projection outputs on the way out.

This version fuses all phases into one software-pipelined instruction
stream to keep ScalarE (the exp bottleneck, ~100us/core) and the PE
(~123us/core) simultaneously busy:
  - x/w_qkv/w_v are uploaded in bf16 (halves input DMA to ~6.5MB);
    DMAs are chunked by xT column-slice and issued in consumption order
    so the first attention exp lands ~8us into the kernel.
  - qk-gen for head pair 0 runs first; v-gen and pair-1 qk-gen chunks
    are interleaved into the attention pipeline of heads 1 and 0
    (sharing one PSUM ring) so the PE never idles long enough for HAM
    to re-throttle the clock.
  - attention per head runs as two window passes g=0/1 (pa [65,1024]
    PSUM x2-ring), per k-block: S^T = kpad_ik @ q^T (runs), P~^T =
    exp(S^T/8) (ScalarE, one op per (ik, 1024-window)), PV accumulated
    into pa with the ones-column denominator trick (row 64).
  - normalization: only the two PSUM->SBUF copies are eager; the
    reciprocal/broadcast/multiply chain (which round-trips SBUF DMAs)
    is deferred and spread over the next head's iterations so it never
    head-of-line-blocks the in-order Vector/GpSimd queues.
  - projection is a 4-deep PSUM pipeline (alternating ring slots) with
    PSUM->SBUF copies alternating Vector/Scalar and per-tile output
    DMAs; m-tiles ordered so the last head's deferred normalize chain
    overlaps the first half of proj.
"""

import numpy as np
from ml_dtypes import bfloat16

import concourse.mybir as mybir
import concourse.tile as tile
from concourse import bacc
from concourse.bass_utils import run_bass_kernel_spmd

B, S, D, H = 2, 2048, 1024, 16
HD = 64          # head dim
BS = 128         # mask block size
NB = S // BS     # 16 blocks per axis
HPC = 4          # heads per core
N_CORES = 8
SCALE = HD ** -0.5
KT = D // 128    # 8 k-tiles over the embedding dim
VW = HPC * (HD + 1)  # 260

F32 = mybir.dt.float32
F32R = mybir.dt.float32r
BF16 = mybir.dt.bfloat16
EXP = mybir.ActivationFunctionType.Exp

_program_cache: dict[bytes, object] = {}


def _plan_runs_g(vis, last_vis, ik, g):
    """Contiguous visible q-block runs for k-block ik within 1024-col
    window g. Runs break at 4-block (512-col = PSUM bank) boundaries."""
    runs = []
    jq, end = 8 * g, 8 * g + 8
    while jq < end:
        if not vis[jq][ik]:
            jq += 1
            continue
        start = jq
        while jq + 1 < end and vis[jq + 1][ik] and (jq + 1) % 4 != 0:
            jq += 1
        stopf = any(last_vis[b] == ik for b in range(start, jq + 1))
        runs.append((start, jq - start + 1, stopf))
        jq += 1
    return runs


def _build_program(mask: np.ndarray):
    vis = [[ik <= jq or not bool(mask[jq, ik]) for ik in range(NB)]
           for jq in range(NB)]
    last_vis = [max(ik for ik in range(NB) if vis[jq][ik]) for jq in range(NB)]
    lastw = [max(last_vis[w * 4:(w + 1) * 4]) for w in range(4)]
    RUNS = {(g, ik): _plan_runs_g(vis, last_vis, ik, g)
            for g in range(2) for ik in range(NB)}

    nc = bacc.Bacc("TRN2", target_bir_lowering=False, debug=False,
                   num_devices=N_CORES)
    xT_d = nc.dram_tensor("xT", [D, S], BF16, kind="ExternalInput")
    # host layout: cols [0:128]=[q0|q1] [128:256]=[k0|k1]
    #              [256:384]=[q2|q3] [384:512]=[k2|k3]
    wqk_d = nc.dram_tensor("wqk", [D, 2 * HPC * HD], BF16, kind="ExternalInput")
    wv_d = nc.dram_tensor("wv", [D, VW], BF16, kind="ExternalInput")
    wpr_d = nc.dram_tensor("wpr", [HPC * HD, D], F32R, kind="ExternalInput")
    out_d = nc.dram_tensor("out", [S, D], BF16, kind="ExternalOutput")

    with tile.TileContext(nc) as tc:
        with tc.tile_pool(name="pp", bufs=1) as pp, \
             tc.tile_pool(name="ptp", bufs=5) as ptp, \
             tc.tile_pool(name="ps", bufs=2, space="PSUM") as ps:
            # ---- persistent SBUF tiles ----
            xT_t = [pp.tile([128, S], BF16, tag=f"xT{k}", name=f"xT{k}")
                    for k in range(KT)]
            wqk_t = [pp.tile([128, 2 * HPC * HD], BF16, tag=f"wqk{k}",
                             name=f"wqk{k}") for k in range(KT)]
            wv_t = [pp.tile([128, VW], BF16, tag=f"wv{k}", name=f"wv{k}")
                    for k in range(KT)]
            wpr_t = [pp.tile([128, D], F32R, tag=f"wpr{k}", name=f"wpr{k}")
                     for k in range(2)]
            q_t = [pp.tile([128, S], F32R, tag=f"q{p}", name=f"q{p}")
                   for p in range(2)]
            kpad_t = [pp.tile([128, S], F32R, tag=f"kp{h}", name=f"kp{h}")
                      for h in range(HPC)]
            v_t = [pp.tile([128, VW], F32R, tag=f"v{m}", name=f"v{m}")
                   for m in range(NB)]
            attn_t = [pp.tile([128, S], F32R, tag=f"attn{i}", name=f"attn{i}")
                      for i in range(2)]
            d16_t = pp.tile([128, 8 * HPC * 2], F32, tag="d16", name="d16")
            r0_t = pp.tile([1, S], F32, tag="r0", name="r0")
            onec = pp.tile([128, 4], F32, tag="onec", name="onec")
            zsrc = pp.tile([64, 512], F32, tag="zsrc", name="zsrc")
            scr = pp.tile([128, 4], F32, tag="scr", name="scr")

            # ---- init + ACT table pre-warm ----
            nc.vector.memset(onec[:], 1.0)
            nc.vector.memset(zsrc[:], 0.0)
            nc.scalar.activation(scr[:], onec[:], EXP, scale=1.0)
            for h in range(HPC):
                z0 = 64 if h % 2 == 0 else 0
                for c in range(4):
                    eng = nc.vector if (h * 4 + c) % 2 == 0 else nc.scalar
                    cs = c * 512
                    if eng is nc.vector:
                        eng.tensor_copy(kpad_t[h][z0:z0 + 64, cs:cs + 512],
                                        zsrc[:])
                    else:
                        eng.copy(kpad_t[h][z0:z0 + 64, cs:cs + 512], zsrc[:])

            # ---- input DMAs in consumption order ----
            # wqk (full); xT cols 0:512, 512:1024; wv; xT cols 1024:2048; wpr
            for k in range(KT):
                nc.sync.dma_start(out=wqk_t[k][:],
                                  in_=wqk_d[k * 128:(k + 1) * 128, :])
            for s in (0, 1):
                for k in range(KT):
                    nc.sync.dma_start(
                        out=xT_t[k][:, s * 512:(s + 1) * 512],
                        in_=xT_d[k * 128:(k + 1) * 128, s * 512:(s + 1) * 512])
            for k in range(KT):
                nc.sync.dma_start(out=wv_t[k][:],
                                  in_=wv_d[k * 128:(k + 1) * 128, :])
            for k in range(KT):
                nc.sync.dma_start(out=xT_t[k][:, 1024:2048],
                                  in_=xT_d[k * 128:(k + 1) * 128, 1024:2048])
            for k in range(2):
                nc.sync.dma_start(out=wpr_t[k][:],
                                  in_=wpr_d[k * 128:(k + 1) * 128, :])

            # ---- gen chunk emitters (copies alternate Vector/Scalar) ----
            genct = [0]

            def qk_chunk(p, t, c, lead=False):
                """[128,512] chunk of q-pair (t=0) or k-pair (t=1) tile."""
                pb = ps.tile([128, 512], F32, tag="st", name=f"pb{p}{t}{c}")
                off = p * 256 + t * 128
                cs = c * 512
                for k in range(KT):
                    nc.tensor.matmul(pb[:], wqk_t[k][:, off:off + 128],
                                     xT_t[k][:, cs:cs + 512],
                                     start=(k == 0), stop=(k == KT - 1))
                genct[0] += 1
                use_sc = genct[0] % 2 == 1
                if t == 0:
                    if use_sc:
                        nc.scalar.copy(q_t[p][:, cs:cs + 512], pb[:])
                    else:
                        nc.vector.tensor_copy(q_t[p][:, cs:cs + 512], pb[:])
                else:
                    h0, h1 = 2 * p, 2 * p + 1
                    if use_sc:
                        nc.scalar.copy(kpad_t[h0][0:64, cs:cs + 512],
                                       pb[0:64, :])
                        nc.vector.tensor_copy(kpad_t[h1][64:128, cs:cs + 512],
                                              pb[64:128, :])
                    else:
                        nc.vector.tensor_copy(kpad_t[h0][0:64, cs:cs + 512],
                                              pb[0:64, :])
                        nc.scalar.copy(kpad_t[h1][64:128, cs:cs + 512],
                                       pb[64:128, :])

            def v_chunk(m):
                pc = ps.tile([128, 512], F32, tag="st", name=f"pc{m}")
                for k in range(KT):
                    nc.tensor.matmul(pc[:, 0:VW],
                                     xT_t[k][:, m * 128:(m + 1) * 128],
                                     wv_t[k][:],
                                     start=(k == 0), stop=(k == KT - 1))
                genct[0] += 1
                if genct[0] % 2 == 1:
                    nc.scalar.copy(v_t[m][:], pc[:, 0:VW])
                else:
                    nc.vector.tensor_copy(v_t[m][:], pc[:, 0:VW])
                nc.vector.tensor_copy(v_t[m][:, HD::HD + 1], onec[:])

            # ---- deferred-op machinery ----
            deferred = []  # [countdown, fn]

            def poll_deferred():
                due = [d for d in deferred if d[0] <= 1]
                for d in due:
                    deferred.remove(d)
                for d in deferred:
                    d[0] -= 1
                for d in due:
                    d[1]()

            def force_deferred(keep=None):
                kept = []
                while deferred:
                    d = deferred.pop(0)
                    if keep is not None and d[2] == keep:
                        kept.append(d)
                    else:
                        d[1]()
                deferred.extend(kept)

            # ---- normalize chain ----
            first_mm = {}   # (j, w) -> True once consumed
            wins_done = {}  # (j, g) -> count

            # per-head staging, ring-allocated (lifetimes span into next head).
            # stage[0:64] = unnormalized attn rows, stage[64:65] = denominator.
            cur = {"stage": None, "odd": None}

            def enqueue_chain(j, g, spacing):
                p, gc = j // 2, g * 1024
                sl = d16_t[:, (2 * j + g) * 8:(2 * j + g + 1) * 8]
                stage, odd = cur["stage"], cur["odd"]
                if j % 2 == 0:
                    dst = attn_t[p][0:64, gc:gc + 1024]
                else:
                    dst = odd[0:64, gc:gc + 1024]

                def s1():
                    nc.gpsimd.dma_start(out=sl, in_=stage[64:65, gc:gc + 1024])

                def s2():
                    nc.vector.reciprocal(sl, sl)

                def s3():
                    nc.gpsimd.dma_start(out=r0_t[0:1, gc:gc + 1024], in_=sl)

                def s4():
                    dbc = pp.tile([64, 1024], F32, tag="dbc", bufs=2,
                                  name=f"dbc{j}{g}")
                    cur[f"dbc{j}{g}"] = dbc
                    nc.gpsimd.partition_broadcast(dbc[:],
                                                  r0_t[0:1, gc:gc + 1024])

                def s5():
                    dbc = cur.pop(f"dbc{j}{g}")
                    nc.vector.tensor_mul(dst, stage[0:64, gc:gc + 1024], dbc[:])

                def s6():
                    nc.gpsimd.dma_start(out=attn_t[p][64:128, gc:gc + 1024],
                                        in_=odd[0:64, gc:gc + 1024])

                steps = [s1, s2, s3, s4, s5] + ([s6] if j % 2 == 1 else [])
                cd = 1
                for i, fn in enumerate(steps):
                    deferred.append([cd, fn, (j, g)])
                    cd += spacing

            def norm_copies(j, g, ik, pa_g):
                for w in (2 * g, 2 * g + 1):
                    if lastw[w] != ik:
                        continue
                    ws = w * 512
                    rel = ws - g * 1024
                    nc.vector.tensor_copy(cur["stage"][0:65, ws:ws + 512],
                                          pa_g[0:65, rel:rel + 512])
                    wins_done[(j, g)] = wins_done.get((j, g), 0) + 1
                    if wins_done[(j, g)] == 2:
                        enqueue_chain(j, g, spacing=3)

            # ---- attention pipeline ----
            pending = [None]  # [(j, g, ik, runs, ptg, pa_g)]

            def flush_pending():
                item = pending[0]
                pending[0] = None
                if item is None:
                    return
                j, g, ik, runs, ptg, pa_g = item
                lhsT_v = v_t[ik][:, j * (HD + 1):(j + 1) * (HD + 1)]
                for (qb0, nbk, stopf) in runs:
                    qs, qlen = qb0 * 128, nbk * 128
                    rel = qs - g * 1024
                    w = qb0 // 4
                    startf = first_mm.pop((j, w), False)
                    nc.tensor.matmul(pa_g[0:65, rel:rel + qlen], lhsT_v,
                                     ptg[:, rel:rel + qlen],
                                     start=startf, stop=stopf,
                                     skip_group_check=True)
                norm_copies(j, g, ik, pa_g)

            def attn_iter(j, g, ik, pa_g, gen=None):
                runs = RUNS[(g, ik)]
                stg = ps.tile([128, 1024], F32, tag="st", name=f"st{j}{g}{ik}")
                lhsT_k = kpad_t[j][:, ik * 128:(ik + 1) * 128]
                qtile = q_t[j // 2]
                for (qb0, nbk, stopf) in runs:
                    qs, qlen = qb0 * 128, nbk * 128
                    rel = qs - g * 1024
                    nc.tensor.matmul(stg[:, rel:rel + qlen], lhsT_k,
                                     qtile[:, qs:qs + qlen],
                                     start=True, stop=True)
                lo = min(r[0] for r in runs) * 128 - g * 1024
                hi = (max(r[0] + r[1] for r in runs)) * 128 - g * 1024
                ptg = ptp.tile([128, 1024], F32R, tag="pt", name=f"pt{j}{g}{ik}")
                nc.scalar.activation(ptg[:, lo:hi], stg[:, lo:hi], EXP,
                                     scale=SCALE)
                if gen is not None:
                    gen()
                poll_deferred()
                flush_pending()
                pending[0] = (j, g, ik, runs, ptg, pa_g)

            # ---- lead: qk-gen for pair 0, windows g=0 ----
            for (t, c) in ((0, 0), (1, 0), (0, 1), (1, 1)):
                qk_chunk(0, t, c, lead=True)

            iters = {g: [ik for ik in range(NB) if RUNS[(g, ik)]]
                     for g in range(2)}

            def head_items(j):
                phases = [(0, [ik for ik in iters[0] if ik < 8]),
                          (1, [ik for ik in iters[1] if ik < 8]),
                          (0, [ik for ik in iters[0] if ik >= 8]),
                          (1, [ik for ik in iters[1] if ik >= 8])]
                return [(g, ik) for (g, iklist) in phases for ik in iklist]

            def edf_schedule(items, chunks):
                """Assign gen chunks to iteration slots by earliest deadline.
                chunks: list of (deadline_slot_inclusive, fn). Returns
                slot -> [fns]; infeasible chunks go to slot 0."""
                slots = {i: [] for i in range(len(items))}
                fill = {i: 0 for i in range(len(items))}
                for dl, fn in sorted(chunks, key=lambda c: c[0]):
                    placed = False
                    # latest-fit: emit just-in-time so gen MMs queue behind
                    # already-arrived DMA data instead of stalling the PE
                    for s in range(min(dl, len(items) - 1), -1, -1):
                        if fill[s] < 2:
                            slots[s].append(fn)
                            fill[s] += 1
                            placed = True
                            break
                    if not placed:
                        slots[0].insert(0, fn)
                return slots

            def head1_chunks(items):
                """v tiles (PV deadline) + pair-0 windows g=1 (QK deadline)."""
                chunks = []
                for m in range(NB):
                    idx = min((i for i, (g, ik) in enumerate(items) if ik == m),
                              default=0)
                    chunks.append((idx + 1, lambda m=m: v_chunk(m)))
                for c in (2, 3):
                    # q chunk c: first QK of window-pair g=c//2 touching it
                    idx = min((i for i, (g, ik) in enumerate(items)
                               if g == c // 2), default=1)
                    chunks.append((max(0, idx - 1),
                                   lambda c=c: qk_chunk(0, 0, c)))
                    # kpad chunk c: first QK with ik in [4c, 4c+4)
                    idx = min((i for i, (g, ik) in enumerate(items)
                               if 4 * c <= ik < 4 * c + 4), default=1)
                    chunks.append((max(0, idx - 1),
                                   lambda c=c: qk_chunk(0, 1, c)))
                return chunks

            def head0_chunks(items):
                """pair-1 gen, needed only by heads 3/2: spread evenly."""
                chunks = []
                pos = 0
                for c in range(4):
                    for t in (0, 1):
                        chunks.append((pos, lambda t=t, c=c: qk_chunk(1, t, c)))
                        pos += 3
                return chunks

            for j in (1, 0, 3, 2):
                items = head_items(j)
                if j == 1:
                    genmap = edf_schedule(items, head1_chunks(items))
                elif j == 0:
                    genmap = edf_schedule(items, head0_chunks(items))
                else:
                    genmap = {}
                for w in range(4):
                    first_mm[(j, w)] = True
                cur["stage"] = pp.tile([65, S], F32, tag="stage", bufs=2,
                                       name=f"stage{j}")
                if j % 2 == 1:
                    cur["odd"] = pp.tile([64, S], F32R, tag="odd", bufs=1,
                                         name=f"odd{j}")
                pa = {}
                for i, (g, ik) in enumerate(items):
                    if g not in pa:
                        pa[g] = ps.tile([65, 1024], F32, tag="pa",
                                        name=f"pa{j}{g}")
                    fns = genmap.get(i, [])
                    gen = (lambda fns=fns: [f() for f in fns]) if fns else None
                    attn_iter(j, g, ik, pa[g], gen=gen)
                flush_pending()
                for w in range(4):
                    first_mm.pop((j, w), None)

            # ---- projection + output ----
            # flush all chains except the last head's g=1 (interleaved below)
            force_deferred(keep=(2, 1))
            last_chain = [d for d in deferred if d[2] == (2, 1)]
            deferred.clear()
            for mi, m in enumerate(list(range(8)) + list(range(8, 16))):
                if last_chain and mi % 2 == 0:
                    last_chain.pop(0)[1]()
                if m == 8:
                    while last_chain:
                        last_chain.pop(0)[1]()
                po = ps.tile([128, D], F32, tag=("st" if mi % 2 == 0 else "pa"),
                             name=f"po{m}")
                for kt in range(2):
                    for c in range(2):
                        nc.tensor.matmul(
                            po[:, c * 512:(c + 1) * 512],
                            attn_t[kt][:, m * 128:(m + 1) * 128],
                            wpr_t[kt][:, c * 512:(c + 1) * 512],
                            start=(kt == 0), stop=(kt == 1))
                ob = pp.tile([128, D], BF16, tag="ob", bufs=3, name=f"ob{m}")
                if mi % 2 == 0:
                    nc.vector.tensor_copy(ob[:], po[:])
                else:
                    nc.scalar.copy(ob[:], po[:])
                # split by partition-halves: two queues drain each tile
                nc.sync.dma_start(out=out_d[m * 128:m * 128 + 64, :],
                                  in_=ob[0:64, :])
                nc.sync.dma_start(out=out_d[m * 128 + 64:(m + 1) * 128, :],
                                  in_=ob[64:128, :])
            while last_chain:
                last_chain.pop(0)[1]()

    # consume first_mm flags at first-visible ik
    nc.compile()
    return nc


def _host_prep(x, w_qkv, w_proj):
    """Per-core input slices. x/wqk/wv in bf16, wpr in f32."""
    xT = [np.ascontiguousarray(x[b].T).astype(bfloat16) for b in range(B)]
    in_maps = []
    for c in range(N_CORES):
        b, grp = c // 4, c % 4
        heads = list(range(grp * HPC, (grp + 1) * HPC))
        wqk = np.empty((D, 2 * HPC * HD), np.float32)
        wv = np.zeros((D, VW), np.float32)
        wpr = np.empty((HPC * HD, D), np.float32)
        for j, h in enumerate(heads):
            p, i = j // 2, j % 2  # pair, index in pair
            # pair block: [q_a|q_b][k_a|k_b] at 256*p
            wqk[:, p * 256 + i * HD:p * 256 + (i + 1) * HD] = \
                w_qkv[:, h * HD:(h + 1) * HD]
            wqk[:, p * 256 + 128 + i * HD:p * 256 + 128 + (i + 1) * HD] = \
                w_qkv[:, D + h * HD:D + (h + 1) * HD]
            wv[:, j * (HD + 1):j * (HD + 1) + HD] = \
                w_qkv[:, 2 * D + h * HD:2 * D + (h + 1) * HD]
            wpr[j * HD:(j + 1) * HD, :] = w_proj[h * HD:(h + 1) * HD, :]
        in_maps.append({
            "xT": xT[b],
            "wqk": np.ascontiguousarray(wqk).astype(bfloat16),
            "wv": np.ascontiguousarray(wv).astype(bfloat16),
            "wpr": np.ascontiguousarray(wpr),
        })
    return in_maps


def get_program(block_mask: np.ndarray):
    key = np.asarray(block_mask, bool).tobytes()
    if key not in _program_cache:
        _program_cache[key] = _build_program(np.asarray(block_mask, bool))
    return _program_cache[key]


def kernel(x, w_qkv, w_proj, b_proj, block_mask):
    x = np.asarray(x, np.float32)
    w_qkv = np.asarray(w_qkv, np.float32)
    w_proj = np.asarray(w_proj, np.float32)
    b_proj = np.asarray(b_proj, np.float32)
    nc = get_program(block_mask)
    in_maps = _host_prep(x, w_qkv, w_proj)
    res = run_bass_kernel_spmd(nc, in_maps, core_ids=list(range(N_CORES)))
    out = np.empty((B, S, D), np.float32)
    for b in range(B):
        acc = np.asarray(res.results[4 * b]["out"], np.float64)
        for g in range(1, 4):
            acc = acc + np.asarray(res.results[4 * b + g]["out"], np.float64)
        out[b] = (acc + b_proj).astype(np.float32)
    return out


# revision 27
# speedup vs baseline: 1.1877x; 1.0266x over previous
"""Block-sparse multi-head attention on 8 Trainium2 NeuronCores.

Problem: y = proj(softmax(mask(q @ k^T / sqrt(hd))) @ v) for
B=2, S=2048, D=1024, H=16 heads, block size 128, with a [16,16] boolean
block mask (True = masked) applied to strictly-upper (k-block > q-block)
blocks.

Sharding: batch x head-group. Core c handles batch c//4 and heads
[4*(c%4), 4*(c%4)+4). No collectives: the host pre-slices inputs
(including pre-transposing x to x^T) and sums the 4 per-batch partial
projection outputs on the way out.

This version fuses all phases into one software-pipelined instruction
stream to keep ScalarE (the exp bottleneck, ~100us/core) and the PE
(~123us/core) simultaneously busy:
  - x/w_qkv/w_v are uploaded in bf16 (halves input DMA to ~6.5MB);
    DMAs are chunked by xT column-slice and issued in consumption order
    so the first attention exp lands ~8us into the kernel.
  - qk-gen for head pair 0 runs first; v-gen and pair-1 qk-gen chunks
    are interleaved into the attention pipeline of heads 1 and 0
    (sharing one PSUM ring) so the PE never idles long enough for HAM
    to re-throttle the clock.
  - attention per head runs as two window passes g=0/1 (pa [65,1024]
    PSUM x2-ring), per k-block: S^T = kpad_ik @ q^T (runs), P~^T =
    exp(S^T/8) (ScalarE, one op per (ik, 1024-window)), PV accumulated
    into pa with the ones-column denominator trick (row 64).
  - normalization: only the two PSUM->SBUF copies are eager; the
    reciprocal/broadcast/multiply chain (which round-trips SBUF DMAs)
    is deferred and spread over the next head's iterations so it never
    head-of-line-blocks the in-order Vector/GpSimd queues.
  - projection is a 4-deep PSUM pipeline (alternating ring slots) with
    PSUM->SBUF copies alternating Vector/Scalar and per-tile output
    DMAs; m-tiles ordered so the last head's deferred normalize chain
    overlaps the first half of proj.
"""

import numpy as np
from ml_dtypes import bfloat16

import concourse.mybir as mybir
import concourse.tile as tile
from concourse import bacc
from concourse.bass_utils import run_bass_kernel_spmd

B, S, D, H = 2, 2048, 1024, 16
HD = 64          # head dim
BS = 128         # mask block size
NB = S // BS     # 16 blocks per axis
HPC = 4          # heads per core
N_CORES = 8
SCALE = HD ** -0.5
KT = D // 128    # 8 k-tiles over the embedding dim
VW = HPC * (HD + 1)  # 260

F32 = mybir.dt.float32
F32R = mybir.dt.float32r
BF16 = mybir.dt.bfloat16
EXP = mybir.ActivationFunctionType.Exp

_program_cache: dict[bytes, object] = {}


def _plan_runs_g(vis, last_vis, ik, g):
    """Contiguous visible q-block runs for k-block ik within 1024-col
    window g. Runs break at 4-block (512-col = PSUM bank) boundaries."""
    runs = []
    jq, end = 8 * g, 8 * g + 8
    while jq < end:
        if not vis[jq][ik]:
            jq += 1
            continue
        start = jq
        while jq + 1 < end and vis[jq + 1][ik] and (jq + 1) % 4 != 0:
            jq += 1
        stopf = any(last_vis[b] == ik for b in range(start, jq + 1))
        runs.append((start, jq - start + 1, stopf))
        jq += 1
    return runs


def _build_program(mask: np.ndarray):
    vis = [[ik <= jq or not bool(mask[jq, ik]) for ik in range(NB)]
           for jq in range(NB)]
    last_vis = [max(ik for ik in range(NB) if vis[jq][ik]) for jq in range(NB)]
    lastw = [max(last_vis[w * 4:(w + 1) * 4]) for w in range(4)]
    RUNS = {(g, ik): _plan_runs_g(vis, last_vis, ik, g)
            for g in range(2) for ik in range(NB)}

    nc = bacc.Bacc("TRN2", target_bir_lowering=False, debug=False,
                   num_devices=N_CORES)
    # host pre-packs everything 128-partition-major and fully contiguous:
    # xT_sl: [128, 4*8*512]  slice-major: slice s (512 seq cols), then k-tile
    # wqk_sl: [128, 8*512]   k-tile major; within: [q0|q1][k0|k1][q2|q3][k2|k3]
    # wv_sl:  [128, 8*260]   k-tile major
    xT_d = nc.dram_tensor("xT", [128, 4 * KT * 512], BF16, kind="ExternalInput")
    wqk_d = nc.dram_tensor("wqk", [128, KT * 512], BF16, kind="ExternalInput")
    wv_d = nc.dram_tensor("wv", [128, KT * VW], BF16, kind="ExternalInput")
    wpr_d = nc.dram_tensor("wpr", [HPC * HD, D], F32R, kind="ExternalInput")
    out_d = nc.dram_tensor("out", [S, D], BF16, kind="ExternalOutput")

    with tile.TileContext(nc) as tc:
        with tc.tile_pool(name="pp", bufs=1) as pp, \
             tc.tile_pool(name="ptp", bufs=5) as ptp, \
             tc.tile_pool(name="ps", bufs=2, space="PSUM") as ps:
            # ---- persistent SBUF tiles ----
            xT_sl = pp.tile([128, 4 * KT * 512], BF16, tag="xT", name="xT")
            wqk_sl = pp.tile([128, KT * 512], BF16, tag="wqk", name="wqk")
            wv_sl = pp.tile([128, KT * VW], BF16, tag="wv", name="wv")
            wpr_t = [pp.tile([128, D], F32R, tag=f"wpr{k}", name=f"wpr{k}")
                     for k in range(2)]
            q_t = [pp.tile([128, S], F32R, tag=f"q{p}", name=f"q{p}")
                   for p in range(2)]
            kpad_t = [pp.tile([128, S], F32R, tag=f"kp{h}", name=f"kp{h}")
                      for h in range(HPC)]
            v_t = [pp.tile([128, VW], F32R, tag=f"v{m}", name=f"v{m}")
                   for m in range(NB)]
            attn_t = [pp.tile([128, S], F32R, tag=f"attn{i}", name=f"attn{i}")
                      for i in range(2)]
            d16_t = pp.tile([128, 8 * HPC * 2], F32, tag="d16", name="d16")
            r0_t = pp.tile([1, S], F32, tag="r0", name="r0")
            onec = pp.tile([128, 4], F32, tag="onec", name="onec")
            zsrc = pp.tile([64, 512], F32, tag="zsrc", name="zsrc")
            scr = pp.tile([128, 4], F32, tag="scr", name="scr")

            # ---- init + ACT table pre-warm ----
            nc.vector.memset(onec[:], 1.0)
            nc.vector.memset(zsrc[:], 0.0)
            nc.scalar.activation(scr[:], onec[:], EXP, scale=1.0)
            for h in range(HPC):
                z0 = 64 if h % 2 == 0 else 0
                for c in range(4):
                    eng = nc.vector if (h * 4 + c) % 2 == 0 else nc.scalar
                    cs = c * 512
                    if eng is nc.vector:
                        eng.tensor_copy(kpad_t[h][z0:z0 + 64, cs:cs + 512],
                                        zsrc[:])
                    else:
                        eng.copy(kpad_t[h][z0:z0 + 64, cs:cs + 512], zsrc[:])

            # ---- input DMAs: contiguous pieces, consumption order ----
            # wqk (8x128KB) | xT s0 (8x128KB) | s1 (8) | wv (2) | s2+s3
            # (8x256KB) | wpr (2x512KB)
            for k in range(KT):
                nc.sync.dma_start(out=wqk_sl[:, k * 512:(k + 1) * 512],
                                  in_=wqk_d[:, k * 512:(k + 1) * 512])
            for s in (0, 1):
                for k in range(KT):
                    o = s * 4096 + k * 512
                    nc.sync.dma_start(out=xT_sl[:, o:o + 512],
                                      in_=xT_d[:, o:o + 512])
            for h in range(2):
                o = h * KT * VW // 2
                nc.sync.dma_start(out=wv_sl[:, o:o + KT * VW // 2],
                                  in_=wv_d[:, o:o + KT * VW // 2])
            for p4 in range(8):
                o = 2 * 4096 + p4 * 1024
                nc.sync.dma_start(out=xT_sl[:, o:o + 1024],
                                  in_=xT_d[:, o:o + 1024])
            for k in range(2):
                nc.sync.dma_start(out=wpr_t[k][:],
                                  in_=wpr_d[k * 128:(k + 1) * 128, :])

            # ---- gen chunk emitters (copies alternate Vector/Scalar) ----
            genct = [0]

            def qk_chunk(p, t, c, lead=False):
                """[128,512] chunk of q-pair (t=0) or k-pair (t=1) tile."""
                pb = ps.tile([128, 512], F32, tag="st", name=f"pb{p}{t}{c}")
                off = p * 256 + t * 128
                cs = c * 512
                for k in range(KT):
                    nc.tensor.matmul(
                        pb[:], wqk_sl[:, k * 512 + off:k * 512 + off + 128],
                        xT_sl[:, c * 4096 + k * 512:c * 4096 + (k + 1) * 512],
                        start=(k == 0), stop=(k == KT - 1))
                genct[0] += 1
                use_sc = genct[0] % 2 == 1
                if t == 0:
                    if use_sc:
                        nc.scalar.copy(q_t[p][:, cs:cs + 512], pb[:])
                    else:
                        nc.vector.tensor_copy(q_t[p][:, cs:cs + 512], pb[:])
                else:
                    h0, h1 = 2 * p, 2 * p + 1
                    if use_sc:
                        nc.scalar.copy(kpad_t[h0][0:64, cs:cs + 512],
                                       pb[0:64, :])
                        nc.vector.tensor_copy(kpad_t[h1][64:128, cs:cs + 512],
                                              pb[64:128, :])
                    else:
                        nc.vector.tensor_copy(kpad_t[h0][0:64, cs:cs + 512],
                                              pb[0:64, :])
                        nc.scalar.copy(kpad_t[h1][64:128, cs:cs + 512],
                                       pb[64:128, :])

            def v_chunk(m):
                pc = ps.tile([128, 512], F32, tag="st", name=f"pc{m}")
                s, r = m // 4, m % 4
                for k in range(KT):
                    nc.tensor.matmul(
                        pc[:, 0:VW],
                        xT_sl[:, s * 4096 + k * 512 + r * 128:
                               s * 4096 + k * 512 + (r + 1) * 128],
                        wv_sl[:, k * VW:(k + 1) * VW],
                        start=(k == 0), stop=(k == KT - 1))
                genct[0] += 1
                if genct[0] % 2 == 1:
                    nc.scalar.copy(v_t[m][:], pc[:, 0:VW])
                else:
                    nc.vector.tensor_copy(v_t[m][:], pc[:, 0:VW])
                nc.vector.tensor_copy(v_t[m][:, HD::HD + 1], onec[:])

            # ---- deferred-op machinery ----
            deferred = []  # [countdown, fn]

            def poll_deferred():
                due = [d for d in deferred if d[0] <= 1]
                for d in due:
                    deferred.remove(d)
                for d in deferred:
                    d[0] -= 1
                for d in due:
                    d[1]()

            def force_deferred(keep=None):
                kept = []
                while deferred:
                    d = deferred.pop(0)
                    if keep is not None and d[2] == keep:
                        kept.append(d)
                    else:
                        d[1]()
                deferred.extend(kept)

            # ---- normalize chain ----
            first_mm = {}   # (j, w) -> True once consumed
            wins_done = {}  # (j, g) -> count

            # per-head staging, ring-allocated (lifetimes span into next head).
            # stage[0:64] = unnormalized attn rows, stage[64:65] = denominator.
            cur = {"stage": None, "odd": None}

            def enqueue_chain(j, g, spacing):
                p, gc = j // 2, g * 1024
                sl = d16_t[:, (2 * j + g) * 8:(2 * j + g + 1) * 8]
                stage, odd = cur["stage"], cur["odd"]
                if j % 2 == 0:
                    dst = attn_t[p][0:64, gc:gc + 1024]
                else:
                    dst = odd[0:64, gc:gc + 1024]

                def s1():
                    nc.gpsimd.dma_start(out=sl, in_=stage[64:65, gc:gc + 1024])

                def s2():
                    nc.vector.reciprocal(sl, sl)

                def s3():
                    nc.gpsimd.dma_start(out=r0_t[0:1, gc:gc + 1024], in_=sl)

                def s4(h):
                    hc = gc + h * 512
                    dbc = pp.tile([64, 512], F32, tag="dbc", bufs=4,
                                  name=f"dbc{j}{g}{h}")
                    cur[f"dbc{j}{g}{h}"] = dbc
                    nc.gpsimd.partition_broadcast(dbc[:],
                                                  r0_t[0:1, hc:hc + 512])

                def s5(h):
                    hc = gc + h * 512
                    dbc = cur.pop(f"dbc{j}{g}{h}")
                    nc.vector.tensor_mul(dst[:, h * 512:(h + 1) * 512],
                                         stage[0:64, hc:hc + 512], dbc[:])

                def s6():
                    nc.gpsimd.dma_start(out=attn_t[p][64:128, gc:gc + 1024],
                                        in_=odd[0:64, gc:gc + 1024])

                steps = [s1, s2, s3,
                         lambda: s4(0), lambda: s5(0),
                         lambda: s4(1), lambda: s5(1)]
                if j % 2 == 1:
                    steps.append(s6)
                cd = 1
                for i, fn in enumerate(steps):
                    deferred.append([cd, fn, (j, g)])
                    cd += spacing

            def norm_copies(j, g, ik, pa_g):
                for w in (2 * g, 2 * g + 1):
                    if lastw[w] != ik:
                        continue
                    ws = w * 512
                    rel = ws - g * 1024
                    nc.vector.tensor_copy(cur["stage"][0:65, ws:ws + 512],
                                          pa_g[0:65, rel:rel + 512])
                    wins_done[(j, g)] = wins_done.get((j, g), 0) + 1
                    if wins_done[(j, g)] == 2:
                        enqueue_chain(j, g, spacing=(1 if j == 2 else 2))

            # ---- attention pipeline ----
            pending = [None]  # [(j, g, ik, runs, ptg, pa_g)]

            def flush_pending():
                item = pending[0]
                pending[0] = None
                if item is None:
                    return
                j, g, ik, runs, ptg, pa_g = item
                lhsT_v = v_t[ik][:, j * (HD + 1):(j + 1) * (HD + 1)]
                for (qb0, nbk, stopf) in runs:
                    qs, qlen = qb0 * 128, nbk * 128
                    rel = qs - g * 1024
                    w = qb0 // 4
                    startf = first_mm.pop((j, w), False)
                    nc.tensor.matmul(pa_g[0:65, rel:rel + qlen], lhsT_v,
                                     ptg[:, rel:rel + qlen],
                                     start=startf, stop=stopf,
                                     skip_group_check=True)
                norm_copies(j, g, ik, pa_g)

            def attn_iter(j, g, ik, pa_g, gen=None):
                runs = RUNS[(g, ik)]
                stg = ps.tile([128, 1024], F32, tag="st", name=f"st{j}{g}{ik}")
                lhsT_k = kpad_t[j][:, ik * 128:(ik + 1) * 128]
                qtile = q_t[j // 2]
                for (qb0, nbk, stopf) in runs:
                    qs, qlen = qb0 * 128, nbk * 128
                    rel = qs - g * 1024
                    nc.tensor.matmul(stg[:, rel:rel + qlen], lhsT_k,
                                     qtile[:, qs:qs + qlen],
                                     start=True, stop=True)
                lo = min(r[0] for r in runs) * 128 - g * 1024
                hi = (max(r[0] + r[1] for r in runs)) * 128 - g * 1024
                ptg = ptp.tile([128, 1024], F32R, tag="pt", name=f"pt{j}{g}{ik}")
                nc.scalar.activation(ptg[:, lo:hi], stg[:, lo:hi], EXP,
                                     scale=SCALE)
                if gen is not None:
                    gen()
                flush_pending()
                poll_deferred()
                pending[0] = (j, g, ik, runs, ptg, pa_g)

            # ---- lead: qk-gen for pair 0, windows g=0 ----
            for (t, c) in ((0, 0), (1, 0), (0, 1), (1, 1)):
                qk_chunk(0, t, c, lead=True)

            iters = {g: [ik for ik in range(NB) if RUNS[(g, ik)]]
                     for g in range(2)}

            def head_items(j):
                phases = [(0, [ik for ik in iters[0] if ik < 8]),
                          (1, [ik for ik in iters[1] if ik < 8]),
                          (0, [ik for ik in iters[0] if ik >= 8]),
                          (1, [ik for ik in iters[1] if ik >= 8])]
                return [(g, ik) for (g, iklist) in phases for ik in iklist]

            def edf_schedule(items, chunks):
                """Assign gen chunks to iteration slots by earliest deadline.
                chunks: list of (deadline_slot_inclusive, fn). Returns
                slot -> [fns]; infeasible chunks go to slot 0."""
                slots = {i: [] for i in range(len(items))}
                fill = {i: 0 for i in range(len(items))}
                for dl, fn in sorted(chunks, key=lambda c: c[0]):
                    placed = False
                    # latest-fit: emit just-in-time so gen MMs queue behind
                    # already-arrived DMA data instead of stalling the PE
                    for s in range(min(dl, len(items) - 1), -1, -1):
                        if fill[s] < 2:
                            slots[s].append(fn)
                            fill[s] += 1
                            placed = True
                            break
                    if not placed:
                        slots[0].insert(0, fn)
                return slots

            def head1_chunks(items):
                """v tiles (PV deadline) + pair-0 windows g=1 (QK deadline)."""
                chunks = []
                for m in range(NB):
                    idx = min((i for i, (g, ik) in enumerate(items) if ik == m),
                              default=0)
                    chunks.append((idx + 1, lambda m=m: v_chunk(m)))
                for c in (2, 3):
                    # q chunk c: first QK of window-pair g=c//2 touching it
                    idx = min((i for i, (g, ik) in enumerate(items)
                               if g == c // 2), default=1)
                    chunks.append((max(0, idx - 1),
                                   lambda c=c: qk_chunk(0, 0, c)))
                    # kpad chunk c: first QK with ik in [4c, 4c+4)
                    idx = min((i for i, (g, ik) in enumerate(items)
                               if 4 * c <= ik < 4 * c + 4), default=1)
                    chunks.append((max(0, idx - 1),
                                   lambda c=c: qk_chunk(0, 1, c)))
                return chunks

            def head0_chunks(items):
                """pair-1 gen, needed only by heads 3/2: spread evenly."""
                chunks = []
                pos = 0
                for c in range(4):
                    for t in (0, 1):
                        chunks.append((pos, lambda t=t, c=c: qk_chunk(1, t, c)))
                        pos += 3
                return chunks

            for j in (1, 0, 3, 2):
                items = head_items(j)
                if j == 1:
                    genmap = edf_schedule(items, head1_chunks(items))
                elif j == 0:
                    genmap = edf_schedule(items, head0_chunks(items))
                else:
                    genmap = {}
                for w in range(4):
                    first_mm[(j, w)] = True
                cur["stage"] = pp.tile([65, S], F32, tag="stage", bufs=2,
                                       name=f"stage{j}")
                if j % 2 == 1:
                    cur["odd"] = pp.tile([64, S], F32R, tag="odd", bufs=1,
                                         name=f"odd{j}")
                pa = {}
                for i, (g, ik) in enumerate(items):
                    if g not in pa:
                        pa[g] = ps.tile([65, 1024], F32, tag="pa",
                                        name=f"pa{j}{g}")
                    fns = genmap.get(i, [])
                    gen = (lambda fns=fns: [f() for f in fns]) if fns else None
                    attn_iter(j, g, ik, pa[g], gen=gen)
                flush_pending()
                for w in range(4):
                    first_mm.pop((j, w), None)

            # ---- projection + output ----
            # flush all chains except the last head's g=1 (interleaved below)
            force_deferred(keep=(2, 1))
            last_chain = [d for d in deferred if d[2] == (2, 1)]
            deferred.clear()
            for mi, m in enumerate(list(range(8)) + list(range(8, 16))):
                if last_chain:
                    last_chain.pop(0)[1]()
                if m == 8:
                    while last_chain:
                        last_chain.pop(0)[1]()
                po = ps.tile([128, D], F32, tag=("st" if mi % 2 == 0 else "pa"),
                             name=f"po{m}")
                for kt in range(2):
                    for c in range(2):
                        nc.tensor.matmul(
                            po[:, c * 512:(c + 1) * 512],
                            attn_t[kt][:, m * 128:(m + 1) * 128],
                            wpr_t[kt][:, c * 512:(c + 1) * 512],
                            start=(kt == 0), stop=(kt == 1))
                ob = pp.tile([128, D], BF16, tag="ob", bufs=3, name=f"ob{m}")
                if mi % 2 == 0:
                    nc.vector.tensor_copy(ob[:], po[:])
                else:
                    nc.scalar.copy(ob[:], po[:])
                # split by partition-halves: two queues drain each tile
                nc.sync.dma_start(out=out_d[m * 128:m * 128 + 64, :],
                                  in_=ob[0:64, :])
                nc.sync.dma_start(out=out_d[m * 128 + 64:(m + 1) * 128, :],
                                  in_=ob[64:128, :])
            while last_chain:
                last_chain.pop(0)[1]()

    # consume first_mm flags at first-visible ik
    nc.compile()
    return nc


def _host_prep(x, w_qkv, w_proj):
    """Per-core input slices, packed 128-partition-major and contiguous.
    x/wqk/wv in bf16, wpr in f32."""
    # xT_sl[b]: [128, 4*8*512] slice-major then k-tile-major
    xT_sl = []
    for b in range(B):
        xT = x[b].T.astype(bfloat16)  # [D, S]
        arr = np.empty((128, 4 * KT * 512), bfloat16)
        for s in range(4):
            for k in range(KT):
                arr[:, s * 4096 + k * 512:s * 4096 + (k + 1) * 512] = \
                    xT[k * 128:(k + 1) * 128, s * 512:(s + 1) * 512]
        xT_sl.append(np.ascontiguousarray(arr))
    in_maps = []
    for c in range(N_CORES):
        b, grp = c // 4, c % 4
        heads = list(range(grp * HPC, (grp + 1) * HPC))
        wqk = np.empty((D, 2 * HPC * HD), np.float32)
        wv = np.zeros((D, VW), np.float32)
        wpr = np.empty((HPC * HD, D), np.float32)
        for j, h in enumerate(heads):
            p, i = j // 2, j % 2  # pair, index in pair
            # pair block: [q_a|q_b][k_a|k_b] at 256*p
            wqk[:, p * 256 + i * HD:p * 256 + (i + 1) * HD] = \
                w_qkv[:, h * HD:(h + 1) * HD]
            wqk[:, p * 256 + 128 + i * HD:p * 256 + 128 + (i + 1) * HD] = \
                w_qkv[:, D + h * HD:D + (h + 1) * HD]
            wv[:, j * (HD + 1):j * (HD + 1) + HD] = \
                w_qkv[:, 2 * D + h * HD:2 * D + (h + 1) * HD]
            wpr[j * HD:(j + 1) * HD, :] = w_proj[h * HD:(h + 1) * HD, :]
        # repack k-tile-major [128, KT*cols]
        wqk_sl = np.empty((128, KT * 512), bfloat16)
        wv_sl = np.empty((128, KT * VW), bfloat16)
        for k in range(KT):
            wqk_sl[:, k * 512:(k + 1) * 512] = \
                wqk[k * 128:(k + 1) * 128, :].astype(bfloat16)
            wv_sl[:, k * VW:(k + 1) * VW] = \
                wv[k * 128:(k + 1) * 128, :].astype(bfloat16)
        in_maps.append({
            "xT": xT_sl[b],
            "wqk": np.ascontiguousarray(wqk_sl),
            "wv": np.ascontiguousarray(wv_sl),
            "wpr": np.ascontiguousarray(wpr),
        })
    return in_maps


def get_program(block_mask: np.ndarray):
    key = np.asarray(block_mask, bool).tobytes()
    if key not in _program_cache:
        _program_cache[key] = _build_program(np.asarray(block_mask, bool))
    return _program_cache[key]


def kernel(x, w_qkv, w_proj, b_proj, block_mask):
    x = np.asarray(x, np.float32)
    w_qkv = np.asarray(w_qkv, np.float32)
    w_proj = np.asarray(w_proj, np.float32)
    b_proj = np.asarray(b_proj, np.float32)
    nc = get_program(block_mask)
    in_maps = _host_prep(x, w_qkv, w_proj)
    res = run_bass_kernel_spmd(nc, in_maps, core_ids=list(range(N_CORES)))
    out = np.empty((B, S, D), np.float32)
    for b in range(B):
        acc = np.asarray(res.results[4 * b]["out"], np.float64)
        for g in range(1, 4):
            acc = acc + np.asarray(res.results[4 * b + g]["out"], np.float64)
        out[b] = (acc + b_proj).astype(np.float32)
    return out


# revision 30
# speedup vs baseline: 1.1914x; 1.0031x over previous
"""Block-sparse multi-head attention on 8 Trainium2 NeuronCores.

Problem: y = proj(softmax(mask(q @ k^T / sqrt(hd))) @ v) for
B=2, S=2048, D=1024, H=16 heads, block size 128, with a [16,16] boolean
block mask (True = masked) applied to strictly-upper (k-block > q-block)
blocks.

Sharding: batch x head-group. Core c handles batch c//4 and heads
[4*(c%4), 4*(c%4)+4). No collectives: the host pre-slices inputs
(including pre-transposing x to x^T) and sums the 4 per-batch partial
projection outputs on the way out.

This version fuses all phases into one software-pipelined instruction
stream to keep ScalarE (the exp bottleneck, ~100us/core) and the PE
(~123us/core) simultaneously busy:
  - x/w_qkv/w_v are uploaded in bf16 (halves input DMA to ~6.5MB);
    DMAs are chunked by xT column-slice and issued in consumption order
    so the first attention exp lands ~8us into the kernel.
  - qk-gen for head pair 0 runs first; v-gen and pair-1 qk-gen chunks
    are interleaved into the attention pipeline of heads 1 and 0
    (sharing one PSUM ring) so the PE never idles long enough for HAM
    to re-throttle the clock.
  - attention per head runs as two window passes g=0/1 (pa [65,1024]
    PSUM x2-ring), per k-block: S^T = kpad_ik @ q^T (runs), P~^T =
    exp(S^T/8) (ScalarE, one op per (ik, 1024-window)), PV accumulated
    into pa with the ones-column denominator trick (row 64).
  - normalization: only the two PSUM->SBUF copies are eager; the
    reciprocal/broadcast/multiply chain (which round-trips SBUF DMAs)
    is deferred and spread over the next head's iterations so it never
    head-of-line-blocks the in-order Vector/GpSimd queues.
  - projection is a 4-deep PSUM pipeline (alternating ring slots) with
    PSUM->SBUF copies alternating Vector/Scalar and per-tile output
    DMAs; m-tiles ordered so the last head's deferred normalize chain
    overlaps the first half of proj.
"""

import numpy as np
from ml_dtypes import bfloat16

import concourse.mybir as mybir
import concourse.tile as tile
from concourse import bacc
from concourse.bass_utils import run_bass_kernel_spmd

B, S, D, H = 2, 2048, 1024, 16
HD = 64          # head dim
BS = 128         # mask block size
NB = S // BS     # 16 blocks per axis
HPC = 4          # heads per core
N_CORES = 8
SCALE = HD ** -0.5
KT = D // 128    # 8 k-tiles over the embedding dim
VW = HPC * (HD + 1)  # 260

F32 = mybir.dt.float32
F32R = mybir.dt.float32r
BF16 = mybir.dt.bfloat16
EXP = mybir.ActivationFunctionType.Exp

_program_cache: dict[bytes, object] = {}


def _plan_runs_g(vis, last_vis, ik, g):
    """Contiguous visible q-block runs for k-block ik within 1024-col
    window g. Runs break at 4-block (512-col = PSUM bank) boundaries."""
    runs = []
    jq, end = 8 * g, 8 * g + 8
    while jq < end:
        if not vis[jq][ik]:
            jq += 1
            continue
        start = jq
        while jq + 1 < end and vis[jq + 1][ik] and (jq + 1) % 4 != 0:
            jq += 1
        stopf = any(last_vis[b] == ik for b in range(start, jq + 1))
        runs.append((start, jq - start + 1, stopf))
        jq += 1
    return runs


def _build_program(mask: np.ndarray):
    vis = [[ik <= jq or not bool(mask[jq, ik]) for ik in range(NB)]
           for jq in range(NB)]
    last_vis = [max(ik for ik in range(NB) if vis[jq][ik]) for jq in range(NB)]
    lastw = [max(last_vis[w * 4:(w + 1) * 4]) for w in range(4)]
    RUNS = {(g, ik): _plan_runs_g(vis, last_vis, ik, g)
            for g in range(2) for ik in range(NB)}

    nc = bacc.Bacc("TRN2", target_bir_lowering=False, debug=False,
                   num_devices=N_CORES)
    # host pre-packs everything 128-partition-major and fully contiguous:
    # xT_sl: [128, 4*8*512]  slice-major: slice s (512 seq cols), then k-tile
    # wqk_sl: [128, 8*512]   k-tile major; within: [q0|q1][k0|k1][q2|q3][k2|k3]
    # wv_sl:  [128, 8*260]   k-tile major
    xT_d = nc.dram_tensor("xT", [128, 4 * KT * 512], BF16, kind="ExternalInput")
    wqk_d = nc.dram_tensor("wqk", [128, KT * 512], BF16, kind="ExternalInput")
    wv_d = nc.dram_tensor("wv", [128, KT * VW], BF16, kind="ExternalInput")
    wpr_d = nc.dram_tensor("wpr", [HPC * HD, D], F32R, kind="ExternalInput")
    out_d = nc.dram_tensor("out", [S, D], BF16, kind="ExternalOutput")

    with tile.TileContext(nc) as tc:
        with tc.tile_pool(name="pp", bufs=1) as pp, \
             tc.tile_pool(name="ptp", bufs=5) as ptp, \
             tc.tile_pool(name="ps", bufs=2, space="PSUM") as ps:
            # ---- persistent SBUF tiles ----
            xT_sl = pp.tile([128, 4 * KT * 512], BF16, tag="xT", name="xT")
            wqk_sl = pp.tile([128, KT * 512], BF16, tag="wqk", name="wqk")
            wv_sl = pp.tile([128, KT * VW], BF16, tag="wv", name="wv")
            wpr_t = [pp.tile([128, D], F32R, tag=f"wpr{k}", name=f"wpr{k}")
                     for k in range(2)]
            q_t = [pp.tile([128, S], F32R, tag=f"q{p}", name=f"q{p}")
                   for p in range(2)]
            kpad_t = [pp.tile([128, S], F32R, tag=f"kp{h}", name=f"kp{h}")
                      for h in range(HPC)]
            v_t = [pp.tile([128, VW], F32R, tag=f"v{m}", name=f"v{m}")
                   for m in range(NB)]
            attn_t = [pp.tile([128, S], F32R, tag=f"attn{i}", name=f"attn{i}")
                      for i in range(2)]
            d16_t = pp.tile([128, 8 * HPC * 2], F32, tag="d16", name="d16")
            r0_t = pp.tile([1, S], F32, tag="r0", name="r0")
            onec = pp.tile([128, 4], F32, tag="onec", name="onec")
            zsrc = pp.tile([64, 512], F32, tag="zsrc", name="zsrc")
            scr = pp.tile([128, 4], F32, tag="scr", name="scr")

            # ---- init + ACT table pre-warm ----
            nc.vector.memset(onec[:], 1.0)
            nc.vector.memset(zsrc[:], 0.0)
            nc.scalar.activation(scr[:], onec[:], EXP, scale=1.0)
            for h in range(HPC):
                z0 = 64 if h % 2 == 0 else 0
                for c in range(4):
                    eng = nc.vector if (h * 4 + c) % 2 == 0 else nc.scalar
                    cs = c * 512
                    if eng is nc.vector:
                        eng.tensor_copy(kpad_t[h][z0:z0 + 64, cs:cs + 512],
                                        zsrc[:])
                    else:
                        eng.copy(kpad_t[h][z0:z0 + 64, cs:cs + 512], zsrc[:])

            # ---- input DMAs: few big contiguous pieces, consumption order,
            # issue alternating between the sync and gpsimd queues (descriptor
            # generation is ~0.6us each and serial per queue) ----
            dmact = [0]

            def in_dma(dst, src):
                eng = nc.sync if dmact[0] % 2 == 0 else nc.gpsimd
                dmact[0] += 1
                eng.dma_start(out=dst, in_=src)

            for h in range(2):  # wqk: 2 x 512KB
                o = h * 2048
                in_dma(wqk_sl[:, o:o + 2048], wqk_d[:, o:o + 2048])
            for s in (0, 1):    # xT s0, s1: 2 x 512KB each
                for h in range(2):
                    o = s * 4096 + h * 2048
                    in_dma(xT_sl[:, o:o + 2048], xT_d[:, o:o + 2048])
            in_dma(wv_sl[:], wv_d[:])  # 530KB
            for p4 in range(4):  # xT s2+s3: 4 x 512KB
                o = 2 * 4096 + p4 * 2048
                in_dma(xT_sl[:, o:o + 2048], xT_d[:, o:o + 2048])
            for k in range(2):
                in_dma(wpr_t[k][:], wpr_d[k * 128:(k + 1) * 128, :])

            # ---- gen chunk emitters (copies alternate Vector/Scalar) ----
            genct = [0]

            def qk_chunk(p, t, c, lead=False):
                """[128,512] chunk of q-pair (t=0) or k-pair (t=1) tile."""
                pb = ps.tile([128, 512], F32, tag="st", name=f"pb{p}{t}{c}")
                off = p * 256 + t * 128
                cs = c * 512
                for k in range(KT):
                    nc.tensor.matmul(
                        pb[:], wqk_sl[:, k * 512 + off:k * 512 + off + 128],
                        xT_sl[:, c * 4096 + k * 512:c * 4096 + (k + 1) * 512],
                        start=(k == 0), stop=(k == KT - 1))
                genct[0] += 1
                use_sc = genct[0] % 2 == 1
                # keep both half-copies of one chunk on ONE engine: the
                # framework serializes sibling readers cross-engine, which
                # couples the exp stream to the Vector queue otherwise
                if t == 0:
                    if use_sc:
                        nc.scalar.copy(q_t[p][:, cs:cs + 512], pb[:])
                    else:
                        nc.vector.tensor_copy(q_t[p][:, cs:cs + 512], pb[:])
                else:
                    h0, h1 = 2 * p, 2 * p + 1
                    if use_sc:
                        nc.scalar.copy(kpad_t[h0][0:64, cs:cs + 512],
                                       pb[0:64, :])
                        nc.scalar.copy(kpad_t[h1][64:128, cs:cs + 512],
                                       pb[64:128, :])
                    else:
                        nc.vector.tensor_copy(kpad_t[h0][0:64, cs:cs + 512],
                                              pb[0:64, :])
                        nc.vector.tensor_copy(kpad_t[h1][64:128, cs:cs + 512],
                                              pb[64:128, :])

            def v_chunk(m):
                pc = ps.tile([128, 512], F32, tag="st", name=f"pc{m}")
                s, r = m // 4, m % 4
                for k in range(KT):
                    nc.tensor.matmul(
                        pc[:, 0:VW],
                        xT_sl[:, s * 4096 + k * 512 + r * 128:
                               s * 4096 + k * 512 + (r + 1) * 128],
                        wv_sl[:, k * VW:(k + 1) * VW],
                        start=(k == 0), stop=(k == KT - 1))
                genct[0] += 1
                if genct[0] % 2 == 1:
                    nc.scalar.copy(v_t[m][:], pc[:, 0:VW])
                else:
                    nc.vector.tensor_copy(v_t[m][:], pc[:, 0:VW])
                nc.vector.tensor_copy(v_t[m][:, HD::HD + 1], onec[:])

            # ---- deferred-op machinery ----
            deferred = []  # [countdown, fn]

            def poll_deferred():
                due = [d for d in deferred if d[0] <= 1]
                for d in due:
                    deferred.remove(d)
                for d in deferred:
                    d[0] -= 1
                for d in due:
                    d[1]()

            def force_deferred(keep=None):
                kept = []
                while deferred:
                    d = deferred.pop(0)
                    if keep is not None and d[2] == keep:
                        kept.append(d)
                    else:
                        d[1]()
                deferred.extend(kept)

            # ---- normalize chain ----
            first_mm = {}   # (j, w) -> True once consumed
            wins_done = {}  # (j, g) -> count

            # per-head staging, ring-allocated (lifetimes span into next head).
            # stage[0:64] = unnormalized attn rows, stage[64:65] = denominator.
            cur = {"stage": None, "odd": None}

            def enqueue_chain(j, g, spacing):
                p, gc = j // 2, g * 1024
                sl = d16_t[:, (2 * j + g) * 8:(2 * j + g + 1) * 8]
                stage, odd = cur["stage"], cur["odd"]
                if j % 2 == 0:
                    dst = attn_t[p][0:64, gc:gc + 1024]
                else:
                    dst = odd[0:64, gc:gc + 1024]

                def s1():
                    nc.gpsimd.dma_start(out=sl, in_=stage[64:65, gc:gc + 1024])

                def s2():
                    nc.vector.reciprocal(sl, sl)

                def s3():
                    nc.gpsimd.dma_start(out=r0_t[0:1, gc:gc + 1024], in_=sl)

                def s4(h):
                    hc = gc + h * 512
                    dbc = pp.tile([64, 512], F32, tag="dbc", bufs=4,
                                  name=f"dbc{j}{g}{h}")
                    cur[f"dbc{j}{g}{h}"] = dbc
                    nc.gpsimd.partition_broadcast(dbc[:],
                                                  r0_t[0:1, hc:hc + 512])

                def s5(h):
                    hc = gc + h * 512
                    dbc = cur.pop(f"dbc{j}{g}{h}")
                    nc.vector.tensor_mul(dst[:, h * 512:(h + 1) * 512],
                                         stage[0:64, hc:hc + 512], dbc[:])

                def s6():
                    nc.gpsimd.dma_start(out=attn_t[p][64:128, gc:gc + 1024],
                                        in_=odd[0:64, gc:gc + 1024])

                steps = [s1, s2, s3,
                         lambda: s4(0), lambda: s5(0),
                         lambda: s4(1), lambda: s5(1)]
                if j % 2 == 1:
                    steps.append(s6)
                cd = 1
                for i, fn in enumerate(steps):
                    deferred.append([cd, fn, (j, g)])
                    cd += spacing

            def norm_copies(j, g, ik, pa_g):
                for w in (2 * g, 2 * g + 1):
                    if lastw[w] != ik:
                        continue
                    ws = w * 512
                    rel = ws - g * 1024
                    nc.vector.tensor_copy(cur["stage"][0:65, ws:ws + 512],
                                          pa_g[0:65, rel:rel + 512])
                    wins_done[(j, g)] = wins_done.get((j, g), 0) + 1
                    if wins_done[(j, g)] == 2:
                        enqueue_chain(j, g, spacing=(1 if j == 2 else 2))

            # ---- attention pipeline ----
            pending = [None]  # [(j, g, ik, runs, ptg, pa_g)]

            def flush_pending():
                item = pending[0]
                pending[0] = None
                if item is None:
                    return
                j, g, ik, runs, ptg, pa_g = item
                lhsT_v = v_t[ik][:, j * (HD + 1):(j + 1) * (HD + 1)]
                for (qb0, nbk, stopf) in runs:
                    qs, qlen = qb0 * 128, nbk * 128
                    rel = qs - g * 1024
                    w = qb0 // 4
                    startf = first_mm.pop((j, w), False)
                    nc.tensor.matmul(pa_g[0:65, rel:rel + qlen], lhsT_v,
                                     ptg[:, rel:rel + qlen],
                                     start=startf, stop=stopf,
                                     skip_group_check=True)
                norm_copies(j, g, ik, pa_g)

            def attn_iter(j, g, ik, pa_g, gen=None):
                runs = RUNS[(g, ik)]
                stg = ps.tile([128, 1024], F32, tag="st", name=f"st{j}{g}{ik}")
                lhsT_k = kpad_t[j][:, ik * 128:(ik + 1) * 128]
                qtile = q_t[j // 2]
                for (qb0, nbk, stopf) in runs:
                    qs, qlen = qb0 * 128, nbk * 128
                    rel = qs - g * 1024
                    nc.tensor.matmul(stg[:, rel:rel + qlen], lhsT_k,
                                     qtile[:, qs:qs + qlen],
                                     start=True, stop=True)
                lo = min(r[0] for r in runs) * 128 - g * 1024
                hi = (max(r[0] + r[1] for r in runs)) * 128 - g * 1024
                ptg = ptp.tile([128, 1024], F32R, tag="pt", name=f"pt{j}{g}{ik}")
                nc.scalar.activation(ptg[:, lo:hi], stg[:, lo:hi], EXP,
                                     scale=SCALE)
                if gen is not None:
                    gen()
                flush_pending()
                poll_deferred()
                pending[0] = (j, g, ik, runs, ptg, pa_g)

            # ---- lead: qk-gen for pair 0, windows g=0 ----
            for (t, c) in ((0, 0), (1, 0), (0, 1), (1, 1)):
                qk_chunk(0, t, c, lead=True)

            iters = {g: [ik for ik in range(NB) if RUNS[(g, ik)]]
                     for g in range(2)}

            def head_items(j):
                phases = [(0, [ik for ik in iters[0] if ik < 8]),
                          (1, [ik for ik in iters[1] if ik < 8]),
                          (0, [ik for ik in iters[0] if ik >= 8]),
                          (1, [ik for ik in iters[1] if ik >= 8])]
                return [(g, ik) for (g, iklist) in phases for ik in iklist]

            def edf_schedule(items, chunks):
                """Assign gen chunks to iteration slots by earliest deadline.
                chunks: list of (deadline_slot_inclusive, fn). Returns
                slot -> [fns]; infeasible chunks go to slot 0."""
                slots = {i: [] for i in range(len(items))}
                fill = {i: 0 for i in range(len(items))}
                for dl, fn in sorted(chunks, key=lambda c: c[0]):
                    placed = False
                    # latest-fit: emit just-in-time so gen MMs queue behind
                    # already-arrived DMA data instead of stalling the PE
                    for s in range(min(dl, len(items) - 1), -1, -1):
                        if fill[s] < 2:
                            slots[s].append(fn)
                            fill[s] += 1
                            placed = True
                            break
                    if not placed:
                        slots[0].insert(0, fn)
                return slots

            def head1_chunks(items):
                """v tiles (PV deadline) + pair-0 windows g=1 (QK deadline)."""
                chunks = []
                for m in range(NB):
                    idx = min((i for i, (g, ik) in enumerate(items) if ik == m),
                              default=0)
                    chunks.append((idx + 1, lambda m=m: v_chunk(m)))
                for c in (2, 3):
                    # q chunk c: first QK of window-pair g=c//2 touching it
                    idx = min((i for i, (g, ik) in enumerate(items)
                               if g == c // 2), default=1)
                    chunks.append((max(0, idx - 1),
                                   lambda c=c: qk_chunk(0, 0, c)))
                    # kpad chunk c: first QK with ik in [4c, 4c+4)
                    idx = min((i for i, (g, ik) in enumerate(items)
                               if 4 * c <= ik < 4 * c + 4), default=1)
                    chunks.append((max(0, idx - 1),
                                   lambda c=c: qk_chunk(0, 1, c)))
                return chunks

            def head0_chunks(items):
                """pair-1 gen, needed only by heads 3/2: spread evenly."""
                chunks = []
                pos = 0
                for c in range(4):
                    for t in (0, 1):
                        chunks.append((pos, lambda t=t, c=c: qk_chunk(1, t, c)))
                        pos += 3
                return chunks

            for j in (1, 0, 3, 2):
                items = head_items(j)
                if j == 1:
                    genmap = edf_schedule(items, head1_chunks(items))
                elif j == 0:
                    genmap = edf_schedule(items, head0_chunks(items))
                else:
                    genmap = {}
                for w in range(4):
                    first_mm[(j, w)] = True
                cur["stage"] = pp.tile([65, S], F32, tag="stage", bufs=2,
                                       name=f"stage{j}")
                if j % 2 == 1:
                    cur["odd"] = pp.tile([64, S], F32R, tag="odd", bufs=1,
                                         name=f"odd{j}")
                pa = {}
                for i, (g, ik) in enumerate(items):
                    if g not in pa:
                        pa[g] = ps.tile([65, 1024], F32, tag="pa",
                                        name=f"pa{j}{g}")
                    fns = genmap.get(i, [])
                    gen = (lambda fns=fns: [f() for f in fns]) if fns else None
                    attn_iter(j, g, ik, pa[g], gen=gen)
                flush_pending()
                for w in range(4):
                    first_mm.pop((j, w), None)

            # ---- projection + output ----
            # flush all chains except the last head's g=1 (interleaved below)
            force_deferred(keep=(2, 1))
            last_chain = [d for d in deferred if d[2] == (2, 1)]
            deferred.clear()
            for mi, m in enumerate(list(range(8)) + list(range(8, 16))):
                if last_chain:
                    last_chain.pop(0)[1]()
                if m == 8:
                    while last_chain:
                        last_chain.pop(0)[1]()
                po = ps.tile([128, D], F32, tag=("st" if mi % 2 == 0 else "pa"),
                             name=f"po{m}")
                for kt in range(2):
                    for c in range(2):
                        nc.tensor.matmul(
                            po[:, c * 512:(c + 1) * 512],
                            attn_t[kt][:, m * 128:(m + 1) * 128],
                            wpr_t[kt][:, c * 512:(c + 1) * 512],
                            start=(kt == 0), stop=(kt == 1))
                ob = pp.tile([128, D], BF16, tag="ob", bufs=3, name=f"ob{m}")
                if mi % 2 == 0:
                    nc.vector.tensor_copy(ob[:], po[:])
                    nc.sync.dma_start(out=out_d[m * 128:(m + 1) * 128, :],
                                      in_=ob[:])
                else:
                    nc.scalar.copy(ob[:], po[:])
                    nc.gpsimd.dma_start(out=out_d[m * 128:(m + 1) * 128, :],
                                        in_=ob[:])
            while last_chain:
                last_chain.pop(0)[1]()

    # consume first_mm flags at first-visible ik
    nc.compile()
    return nc


def _host_prep(x, w_qkv, w_proj):
    """Per-core input slices, packed 128-partition-major and contiguous.
    x/wqk/wv in bf16, wpr in f32."""
    # xT_sl[b]: [128, 4*8*512] slice-major then k-tile-major
    xT_sl = []
    for b in range(B):
        xT = x[b].T.astype(bfloat16)  # [D, S]
        arr = np.empty((128, 4 * KT * 512), bfloat16)
        for s in range(4):
            for k in range(KT):
                arr[:, s * 4096 + k * 512:s * 4096 + (k + 1) * 512] = \
                    xT[k * 128:(k + 1) * 128, s * 512:(s + 1) * 512]
        xT_sl.append(np.ascontiguousarray(arr))
    in_maps = []
    for c in range(N_CORES):
        b, grp = c // 4, c % 4
        heads = list(range(grp * HPC, (grp + 1) * HPC))
        wqk = np.empty((D, 2 * HPC * HD), np.float32)
        wv = np.zeros((D, VW), np.float32)
        wpr = np.empty((HPC * HD, D), np.float32)
        for j, h in enumerate(heads):
            p, i = j // 2, j % 2  # pair, index in pair
            # pair block: [q_a|q_b][k_a|k_b] at 256*p
            wqk[:, p * 256 + i * HD:p * 256 + (i + 1) * HD] = \
                w_qkv[:, h * HD:(h + 1) * HD]
            wqk[:, p * 256 + 128 + i * HD:p * 256 + 128 + (i + 1) * HD] = \
                w_qkv[:, D + h * HD:D + (h + 1) * HD]
            wv[:, j * (HD + 1):j * (HD + 1) + HD] = \
                w_qkv[:, 2 * D + h * HD:2 * D + (h + 1) * HD]
            wpr[j * HD:(j + 1) * HD, :] = w_proj[h * HD:(h + 1) * HD, :]
        # repack k-tile-major [128, KT*cols]
        wqk_sl = np.empty((128, KT * 512), bfloat16)
        wv_sl = np.empty((128, KT * VW), bfloat16)
        for k in range(KT):
            wqk_sl[:, k * 512:(k + 1) * 512] = \
                wqk[k * 128:(k + 1) * 128, :].astype(bfloat16)
            wv_sl[:, k * VW:(k + 1) * VW] = \
                wv[k * 128:(k + 1) * 128, :].astype(bfloat16)
        in_maps.append({
            "xT": xT_sl[b],
            "wqk": np.ascontiguousarray(wqk_sl),
            "wv": np.ascontiguousarray(wv_sl),
            "wpr": np.ascontiguousarray(wpr),
        })
    return in_maps


def get_program(block_mask: np.ndarray):
    key = np.asarray(block_mask, bool).tobytes()
    if key not in _program_cache:
        _program_cache[key] = _build_program(np.asarray(block_mask, bool))
    return _program_cache[key]


def kernel(x, w_qkv, w_proj, b_proj, block_mask):
    x = np.asarray(x, np.float32)
    w_qkv = np.asarray(w_qkv, np.float32)
    w_proj = np.asarray(w_proj, np.float32)
    b_proj = np.asarray(b_proj, np.float32)
    nc = get_program(block_mask)
    in_maps = _host_prep(x, w_qkv, w_proj)
    res = run_bass_kernel_spmd(nc, in_maps, core_ids=list(range(N_CORES)))
    out = np.empty((B, S, D), np.float32)
    for b in range(B):
        acc = np.asarray(res.results[4 * b]["out"], np.float64)
        for g in range(1, 4):
            acc = acc + np.asarray(res.results[4 * b + g]["out"], np.float64)
        out[b] = (acc + b_proj).astype(np.float32)
    return out


# revision 33
# speedup vs baseline: 1.3226x; 1.1101x over previous
"""Block-sparse multi-head attention on 8 Trainium2 NeuronCores.

Problem: y = proj(softmax(mask(q @ k^T / sqrt(hd))) @ v) for
B=2, S=2048, D=1024, H=16 heads, block size 128, with a [16,16] boolean
block mask (True = masked) applied to strictly-upper (k-block > q-block)
blocks.

Sharding: batch x head-group. Core c handles batch c//4 and heads
[4*(c%4), 4*(c%4)+4). No collectives: the host pre-slices inputs
(including pre-transposing x to x^T) and sums the 4 per-batch partial
projection outputs on the way out.

This version fuses all phases into one software-pipelined instruction
stream to keep ScalarE (the exp bottleneck, ~100us/core) and the PE
(~123us/core) simultaneously busy:
  - x/w_qkv/w_v are uploaded in bf16 (halves input DMA to ~6.5MB);
    DMAs are chunked by xT column-slice and issued in consumption order
    so the first attention exp lands ~8us into the kernel.
  - qk-gen for head pair 0 runs first; v-gen and pair-1 qk-gen chunks
    are interleaved into the attention pipeline of heads 1 and 0
    (sharing one PSUM ring) so the PE never idles long enough for HAM
    to re-throttle the clock.
  - attention per head runs as two window passes g=0/1 (pa [65,1024]
    PSUM x2-ring), per k-block: S^T = kpad_ik @ q^T (runs), P~^T =
    exp(S^T/8) (ScalarE, one op per (ik, 1024-window)), PV accumulated
    into pa with the ones-column denominator trick (row 64).
  - normalization: only the two PSUM->SBUF copies are eager; the
    reciprocal/broadcast/multiply chain (which round-trips SBUF DMAs)
    is deferred and spread over the next head's iterations so it never
    head-of-line-blocks the in-order Vector/GpSimd queues.
  - projection is a 4-deep PSUM pipeline (alternating ring slots) with
    PSUM->SBUF copies alternating Vector/Scalar and per-tile output
    DMAs; m-tiles ordered so the last head's deferred normalize chain
    overlaps the first half of proj.
"""

import numpy as np
from ml_dtypes import bfloat16

import concourse.mybir as mybir
import concourse.tile as tile
from concourse import bacc
from concourse.bass_utils import run_bass_kernel_spmd

B, S, D, H = 2, 2048, 1024, 16
HD = 64          # head dim
BS = 128         # mask block size
NB = S // BS     # 16 blocks per axis
HPC = 4          # heads per core
N_CORES = 8
SCALE = HD ** -0.5
KT = D // 128    # 8 k-tiles over the embedding dim
VW = HPC * (HD + 1)  # 260

F32 = mybir.dt.float32
F32R = mybir.dt.float32r
BF16 = mybir.dt.bfloat16
EXP = mybir.ActivationFunctionType.Exp

_program_cache: dict[bytes, object] = {}


def _plan_runs_g(vis, last_vis, ik, g):
    """Contiguous visible q-block runs for k-block ik within 1024-col
    window g. Runs break at 4-block (512-col = PSUM bank) boundaries."""
    runs = []
    jq, end = 8 * g, 8 * g + 8
    while jq < end:
        if not vis[jq][ik]:
            jq += 1
            continue
        start = jq
        while jq + 1 < end and vis[jq + 1][ik] and (jq + 1) % 4 != 0:
            jq += 1
        stopf = any(last_vis[b] == ik for b in range(start, jq + 1))
        runs.append((start, jq - start + 1, stopf))
        jq += 1
    return runs


def _build_program(mask: np.ndarray):
    vis = [[ik <= jq or not bool(mask[jq, ik]) for ik in range(NB)]
           for jq in range(NB)]
    last_vis = [max(ik for ik in range(NB) if vis[jq][ik]) for jq in range(NB)]
    lastw = [max(last_vis[w * 4:(w + 1) * 4]) for w in range(4)]
    RUNS = {(g, ik): _plan_runs_g(vis, last_vis, ik, g)
            for g in range(2) for ik in range(NB)}

    nc = bacc.Bacc("TRN2", target_bir_lowering=False, debug=False,
                   num_devices=N_CORES)
    # host pre-packs everything 128-partition-major and fully contiguous:
    # xT_sl: [128, 4*8*512]  slice-major: slice s (512 seq cols), then k-tile
    # wqk_sl: [128, 8*512]   k-tile major; within: [q0|q1][k0|k1][q2|q3][k2|k3]
    # wv_sl:  [128, 8*260]   k-tile major
    xT_d = nc.dram_tensor("xT", [128, 4 * KT * 512], BF16, kind="ExternalInput")
    wqk_d = nc.dram_tensor("wqk", [128, KT * 512], BF16, kind="ExternalInput")
    wv_d = nc.dram_tensor("wv", [128, KT * VW], BF16, kind="ExternalInput")
    wpr_d = nc.dram_tensor("wpr", [HPC * HD, D], F32R, kind="ExternalInput")
    out_d = nc.dram_tensor("out", [S, D], BF16, kind="ExternalOutput")

    with tile.TileContext(nc) as tc:
        with tc.tile_pool(name="pp", bufs=1) as pp, \
             tc.tile_pool(name="ptp", bufs=5) as ptp, \
             tc.tile_pool(name="ps", bufs=2, space="PSUM") as ps:
            # ---- persistent SBUF tiles ----
            xT_sl = pp.tile([128, 4 * KT * 512], BF16, tag="xT", name="xT")
            wqk_sl = pp.tile([128, KT * 512], BF16, tag="wqk", name="wqk")
            wv_sl = pp.tile([128, KT * VW], BF16, tag="wv", name="wv")
            wpr_t = [pp.tile([128, D], F32R, tag=f"wpr{k}", name=f"wpr{k}")
                     for k in range(2)]
            q_t = [pp.tile([128, S], F32R, tag=f"q{p}", name=f"q{p}")
                   for p in range(2)]
            kpad_t = [pp.tile([128, S], F32R, tag=f"kp{h}", name=f"kp{h}")
                      for h in range(HPC)]
            v_t = [pp.tile([128, VW], F32R, tag=f"v{m}", name=f"v{m}")
                   for m in range(NB)]
            attn_t = [pp.tile([128, S], F32R, tag=f"attn{i}", name=f"attn{i}")
                      for i in range(2)]
            d16_t = pp.tile([128, 8 * HPC * 2], F32, tag="d16", name="d16")
            r0_t = pp.tile([1, S], F32, tag="r0", name="r0")
            onec = pp.tile([128, 4], F32, tag="onec", name="onec")
            zsrc = pp.tile([64, 512], F32, tag="zsrc", name="zsrc")
            scr = pp.tile([128, 4], F32, tag="scr", name="scr")

            # ---- init + ACT table pre-warm ----
            nc.vector.memset(onec[:], 1.0)
            nc.vector.memset(zsrc[:], 0.0)
            nc.scalar.activation(scr[:], onec[:], EXP, scale=1.0)
            for h in range(HPC):
                z0 = 64 if h % 2 == 0 else 0
                for c in range(4):
                    eng = nc.vector if (h * 4 + c) % 2 == 0 else nc.scalar
                    cs = c * 512
                    if eng is nc.vector:
                        eng.tensor_copy(kpad_t[h][z0:z0 + 64, cs:cs + 512],
                                        zsrc[:])
                    else:
                        eng.copy(kpad_t[h][z0:z0 + 64, cs:cs + 512], zsrc[:])

            # ---- input DMAs: few big contiguous pieces, consumption order,
            # issue alternating between the sync and gpsimd queues (descriptor
            # generation is ~0.6us each and serial per queue) ----
            dmact = [0]

            def in_dma(dst, src):
                eng = nc.sync if dmact[0] % 2 == 0 else nc.gpsimd
                dmact[0] += 1
                eng.dma_start(out=dst, in_=src)

            for h in range(2):  # wqk: 2 x 512KB
                o = h * 2048
                in_dma(wqk_sl[:, o:o + 2048], wqk_d[:, o:o + 2048])
            for s in (0, 1):    # xT s0, s1: 2 x 512KB each
                for h in range(2):
                    o = s * 4096 + h * 2048
                    in_dma(xT_sl[:, o:o + 2048], xT_d[:, o:o + 2048])
            in_dma(wv_sl[:], wv_d[:])  # 530KB
            for p4 in range(4):  # xT s2+s3: 4 x 512KB
                o = 2 * 4096 + p4 * 2048
                in_dma(xT_sl[:, o:o + 2048], xT_d[:, o:o + 2048])
            for k in range(2):
                in_dma(wpr_t[k][:], wpr_d[k * 128:(k + 1) * 128, :])

            # ---- gen chunk emitters (copies alternate Vector/Scalar) ----
            genct = [0]

            def qk_chunk(p, t, c, lead=False):
                """[128,512] chunk of q-pair (t=0) or k-pair (t=1) tile."""
                pb = ps.tile([128, 512], F32, tag="st", bufs=3, name=f"pb{p}{t}{c}")
                off = p * 256 + t * 128
                cs = c * 512
                for k in range(KT):
                    nc.tensor.matmul(
                        pb[:], wqk_sl[:, k * 512 + off:k * 512 + off + 128],
                        xT_sl[:, c * 4096 + k * 512:c * 4096 + (k + 1) * 512],
                        start=(k == 0), stop=(k == KT - 1))
                genct[0] += 1
                use_sc = genct[0] % 2 == 1
                # keep both half-copies of one chunk on ONE engine: the
                # framework serializes sibling readers cross-engine, which
                # couples the exp stream to the Vector queue otherwise
                if t == 0:
                    if use_sc:
                        nc.scalar.copy(q_t[p][:, cs:cs + 512], pb[:])
                    else:
                        nc.vector.tensor_copy(q_t[p][:, cs:cs + 512], pb[:])
                else:
                    h0, h1 = 2 * p, 2 * p + 1
                    if use_sc:
                        nc.scalar.copy(kpad_t[h0][0:64, cs:cs + 512],
                                       pb[0:64, :])
                        nc.scalar.copy(kpad_t[h1][64:128, cs:cs + 512],
                                       pb[64:128, :])
                    else:
                        nc.vector.tensor_copy(kpad_t[h0][0:64, cs:cs + 512],
                                              pb[0:64, :])
                        nc.vector.tensor_copy(kpad_t[h1][64:128, cs:cs + 512],
                                              pb[64:128, :])

            def v_chunk(m):
                pc = ps.tile([128, 512], F32, tag="st", bufs=3, name=f"pc{m}")
                s, r = m // 4, m % 4
                for k in range(KT):
                    nc.tensor.matmul(
                        pc[:, 0:VW],
                        xT_sl[:, s * 4096 + k * 512 + r * 128:
                               s * 4096 + k * 512 + (r + 1) * 128],
                        wv_sl[:, k * VW:(k + 1) * VW],
                        start=(k == 0), stop=(k == KT - 1))
                genct[0] += 1
                if genct[0] % 2 == 1:
                    nc.scalar.copy(v_t[m][:], pc[:, 0:VW])
                else:
                    nc.vector.tensor_copy(v_t[m][:], pc[:, 0:VW])
                nc.vector.tensor_copy(v_t[m][:, HD::HD + 1], onec[:])

            # ---- deferred-op machinery ----
            deferred = []  # [countdown, fn]

            def poll_deferred():
                due = [d for d in deferred if d[0] <= 1]
                for d in due:
                    deferred.remove(d)
                for d in deferred:
                    d[0] -= 1
                for d in due:
                    d[1]()

            def force_deferred(keep=None):
                kept = []
                while deferred:
                    d = deferred.pop(0)
                    if keep is not None and d[2] == keep:
                        kept.append(d)
                    else:
                        d[1]()
                deferred.extend(kept)

            # ---- normalize chain ----
            first_mm = {}   # (j, w) -> True once consumed
            wins_done = {}  # (j, g) -> count

            # per-head staging, ring-allocated (lifetimes span into next head).
            # stage[0:64] = unnormalized attn rows, stage[64:65] = denominator.
            cur = {"stage": None, "odd": None}

            def enqueue_chain(j, g, spacing):
                p, gc = j // 2, g * 1024
                sl = d16_t[:, (2 * j + g) * 8:(2 * j + g + 1) * 8]
                stage, odd = cur["stage"], cur["odd"]
                if j % 2 == 0:
                    dst = attn_t[p][0:64, gc:gc + 1024]
                else:
                    dst = odd[0:64, gc:gc + 1024]

                def s1():
                    nc.gpsimd.dma_start(out=sl, in_=stage[64:65, gc:gc + 1024])

                def s2():
                    nc.vector.reciprocal(sl, sl)

                def s3():
                    nc.gpsimd.dma_start(out=r0_t[0:1, gc:gc + 1024], in_=sl)

                def s4(h):
                    hc = gc + h * 512
                    dbc = pp.tile([64, 512], F32, tag="dbc", bufs=4,
                                  name=f"dbc{j}{g}{h}")
                    cur[f"dbc{j}{g}{h}"] = dbc
                    nc.gpsimd.partition_broadcast(dbc[:],
                                                  r0_t[0:1, hc:hc + 512])

                def s5(h):
                    hc = gc + h * 512
                    dbc = cur.pop(f"dbc{j}{g}{h}")
                    nc.vector.tensor_mul(dst[:, h * 512:(h + 1) * 512],
                                         stage[0:64, hc:hc + 512], dbc[:])

                def s6():
                    nc.gpsimd.dma_start(out=attn_t[p][64:128, gc:gc + 1024],
                                        in_=odd[0:64, gc:gc + 1024])

                # both PBs issued before the MULs so the GpSimd latency is
                # hidden before the Vector ops need the result
                steps = [(1, s1), (2, s2), (1, s3),
                         (1, lambda: s4(0)), (1, lambda: s4(1)),
                         (2, lambda: s5(0)), (1, lambda: s5(1))]
                if j % 2 == 1:
                    steps.append((1, s6))
                cd = 0
                for extra, fn in steps:
                    cd += spacing * extra
                    deferred.append([cd, fn, (j, g)])

            def norm_copies(j, g, ik, pa_g):
                for w in (2 * g, 2 * g + 1):
                    if lastw[w] != ik:
                        continue
                    ws = w * 512
                    rel = ws - g * 1024
                    nc.vector.tensor_copy(cur["stage"][0:65, ws:ws + 512],
                                          pa_g[0:65, rel:rel + 512])
                    wins_done[(j, g)] = wins_done.get((j, g), 0) + 1
                    if wins_done[(j, g)] == 2:
                        enqueue_chain(j, g, spacing=(1 if j == 2 else 2))

            # ---- attention pipeline ----
            pending = [None]  # [(j, g, ik, runs, ptg, pa_g)]

            def flush_pending():
                item = pending[0]
                pending[0] = None
                if item is None:
                    return
                j, g, ik, runs, ptg, pa_g = item
                lhsT_v = v_t[ik][:, j * (HD + 1):(j + 1) * (HD + 1)]
                for (qb0, nbk, stopf) in runs:
                    qs, qlen = qb0 * 128, nbk * 128
                    rel = qs - g * 1024
                    w = qb0 // 4
                    startf = first_mm.pop((j, w), False)
                    nc.tensor.matmul(pa_g[0:65, rel:rel + qlen], lhsT_v,
                                     ptg[:, rel:rel + qlen],
                                     start=startf, stop=stopf,
                                     skip_group_check=True)
                norm_copies(j, g, ik, pa_g)

            def attn_iter(j, g, ik, pa_g, gen=None):
                runs = RUNS[(g, ik)]
                stg = ps.tile([128, 1024], F32, tag="st", bufs=3, name=f"st{j}{g}{ik}")
                lhsT_k = kpad_t[j][:, ik * 128:(ik + 1) * 128]
                qtile = q_t[j // 2]
                for (qb0, nbk, stopf) in runs:
                    qs, qlen = qb0 * 128, nbk * 128
                    rel = qs - g * 1024
                    nc.tensor.matmul(stg[:, rel:rel + qlen], lhsT_k,
                                     qtile[:, qs:qs + qlen],
                                     start=True, stop=True)
                lo = min(r[0] for r in runs) * 128 - g * 1024
                hi = (max(r[0] + r[1] for r in runs)) * 128 - g * 1024
                ptg = ptp.tile([128, 1024], F32R, tag="pt", name=f"pt{j}{g}{ik}")
                nc.scalar.activation(ptg[:, lo:hi], stg[:, lo:hi], EXP,
                                     scale=SCALE)
                if gen is not None:
                    gen()
                flush_pending()
                poll_deferred()
                pending[0] = (j, g, ik, runs, ptg, pa_g)

            # ---- lead: qk-gen for pair 0, windows g=0 ----
            for (t, c) in ((0, 0), (1, 0), (0, 1), (1, 1)):
                qk_chunk(0, t, c, lead=True)

            iters = {g: [ik for ik in range(NB) if RUNS[(g, ik)]]
                     for g in range(2)}

            def head_items(j):
                # g=0 fully then g=1: only ONE pa tile live at a time, which
                # frees 2 PSUM banks for a 3-deep stg ring (the exp pipeline)
                phases = [(0, iters[0]), (1, iters[1])]
                return [(g, ik) for (g, iklist) in phases for ik in iklist]

            def edf_schedule(items, chunks):
                """Assign gen chunks to iteration slots by earliest deadline.
                chunks: list of (deadline_slot_inclusive, fn). Returns
                slot -> [fns]; infeasible chunks go to slot 0."""
                slots = {i: [] for i in range(len(items))}
                fill = {i: 0 for i in range(len(items))}
                for dl, fn in sorted(chunks, key=lambda c: c[0]):
                    placed = False
                    # latest-fit: emit just-in-time so gen MMs queue behind
                    # already-arrived DMA data instead of stalling the PE
                    for s in range(min(dl, len(items) - 1), -1, -1):
                        if fill[s] < 2:
                            slots[s].append(fn)
                            fill[s] += 1
                            placed = True
                            break
                    if not placed:
                        slots[0].insert(0, fn)
                return slots

            def head1_chunks(items):
                """v tiles (PV deadline) + pair-0 windows g=1 (QK deadline)."""
                chunks = []
                for m in range(NB):
                    idx = min((i for i, (g, ik) in enumerate(items) if ik == m),
                              default=0)
                    chunks.append((idx + 1, lambda m=m: v_chunk(m)))
                for c in (2, 3):
                    # q chunk c: first QK of window-pair g=c//2 touching it
                    idx = min((i for i, (g, ik) in enumerate(items)
                               if g == c // 2), default=1)
                    chunks.append((max(0, idx - 1),
                                   lambda c=c: qk_chunk(0, 0, c)))
                    # kpad chunk c: first QK with ik in [4c, 4c+4)
                    idx = min((i for i, (g, ik) in enumerate(items)
                               if 4 * c <= ik < 4 * c + 4), default=1)
                    chunks.append((max(0, idx - 1),
                                   lambda c=c: qk_chunk(0, 1, c)))
                return chunks

            def head0_chunks(items):
                """pair-1 gen, needed only by heads 3/2: spread evenly."""
                chunks = []
                pos = 0
                for c in range(4):
                    for t in (0, 1):
                        chunks.append((pos, lambda t=t, c=c: qk_chunk(1, t, c)))
                        pos += 3
                return chunks

            for j in (1, 0, 3, 2):
                items = head_items(j)
                if j == 1:
                    genmap = edf_schedule(items, head1_chunks(items))
                elif j == 0:
                    genmap = edf_schedule(items, head0_chunks(items))
                else:
                    genmap = {}
                for w in range(4):
                    first_mm[(j, w)] = True
                cur["stage"] = pp.tile([65, S], F32, tag="stage", bufs=2,
                                       name=f"stage{j}")
                if j % 2 == 1:
                    cur["odd"] = pp.tile([64, S], F32R, tag="odd", bufs=1,
                                         name=f"odd{j}")
                pa = {}
                for i, (g, ik) in enumerate(items):
                    if g not in pa:
                        pa[g] = ps.tile([65, 1024], F32, tag="pa", bufs=1,
                                        name=f"pa{j}{g}")
                    fns = genmap.get(i, [])
                    gen = (lambda fns=fns: [f() for f in fns]) if fns else None
                    attn_iter(j, g, ik, pa[g], gen=gen)
                flush_pending()
                for w in range(4):
                    first_mm.pop((j, w), None)

            # ---- projection + output ----
            # flush all chains except the last head's g=1 (interleaved below)
            force_deferred(keep=(2, 1))
            last_chain = [d for d in deferred if d[2] == (2, 1)]
            deferred.clear()
            for mi, m in enumerate(list(range(8)) + list(range(8, 16))):
                if last_chain:
                    last_chain.pop(0)[1]()
                if m == 8:
                    while last_chain:
                        last_chain.pop(0)[1]()
                po = ps.tile([128, D], F32, tag=("st" if mi % 2 == 0 else "pa"),
                             bufs=(3 if mi % 2 == 0 else 1), name=f"po{m}")
                for kt in range(2):
                    for c in range(2):
                        nc.tensor.matmul(
                            po[:, c * 512:(c + 1) * 512],
                            attn_t[kt][:, m * 128:(m + 1) * 128],
                            wpr_t[kt][:, c * 512:(c + 1) * 512],
                            start=(kt == 0), stop=(kt == 1))
                ob = pp.tile([128, D], BF16, tag="ob", bufs=3, name=f"ob{m}")
                if mi % 2 == 0:
                    nc.vector.tensor_copy(ob[:], po[:])
                    nc.sync.dma_start(out=out_d[m * 128:(m + 1) * 128, :],
                                      in_=ob[:])
                else:
                    nc.scalar.copy(ob[:], po[:])
                    nc.gpsimd.dma_start(out=out_d[m * 128:(m + 1) * 128, :],
                                        in_=ob[:])
            while last_chain:
                last_chain.pop(0)[1]()

    # consume first_mm flags at first-visible ik
    nc.compile()
    return nc


def _host_prep(x, w_qkv, w_proj):
    """Per-core input slices, packed 128-partition-major and contiguous.
    x/wqk/wv in bf16, wpr in f32."""
    # xT_sl[b]: [128, 4*8*512] slice-major then k-tile-major
    xT_sl = []
    for b in range(B):
        xT = x[b].T.astype(bfloat16)  # [D, S]
        arr = np.empty((128, 4 * KT * 512), bfloat16)
        for s in range(4):
            for k in range(KT):
                arr[:, s * 4096 + k * 512:s * 4096 + (k + 1) * 512] = \
                    xT[k * 128:(k + 1) * 128, s * 512:(s + 1) * 512]
        xT_sl.append(np.ascontiguousarray(arr))
    in_maps = []
    for c in range(N_CORES):
        b, grp = c // 4, c % 4
        heads = list(range(grp * HPC, (grp + 1) * HPC))
        wqk = np.empty((D, 2 * HPC * HD), np.float32)
        wv = np.zeros((D, VW), np.float32)
        wpr = np.empty((HPC * HD, D), np.float32)
        for j, h in enumerate(heads):
            p, i = j // 2, j % 2  # pair, index in pair
            # pair block: [q_a|q_b][k_a|k_b] at 256*p
            wqk[:, p * 256 + i * HD:p * 256 + (i + 1) * HD] = \
                w_qkv[:, h * HD:(h + 1) * HD]
            wqk[:, p * 256 + 128 + i * HD:p * 256 + 128 + (i + 1) * HD] = \
                w_qkv[:, D + h * HD:D + (h + 1) * HD]
            wv[:, j * (HD + 1):j * (HD + 1) + HD] = \
                w_qkv[:, 2 * D + h * HD:2 * D + (h + 1) * HD]
            wpr[j * HD:(j + 1) * HD, :] = w_proj[h * HD:(h + 1) * HD, :]
        # repack k-tile-major [128, KT*cols]
        wqk_sl = np.empty((128, KT * 512), bfloat16)
        wv_sl = np.empty((128, KT * VW), bfloat16)
        for k in range(KT):
            wqk_sl[:, k * 512:(k + 1) * 512] = \
                wqk[k * 128:(k + 1) * 128, :].astype(bfloat16)
            wv_sl[:, k * VW:(k + 1) * VW] = \
                wv[k * 128:(k + 1) * 128, :].astype(bfloat16)
        in_maps.append({
            "xT": xT_sl[b],
            "wqk": np.ascontiguousarray(wqk_sl),
            "wv": np.ascontiguousarray(wv_sl),
            "wpr": np.ascontiguousarray(wpr),
        })
    return in_maps


def get_program(block_mask: np.ndarray):
    key = np.asarray(block_mask, bool).tobytes()
    if key not in _program_cache:
        _program_cache[key] = _build_program(np.asarray(block_mask, bool))
    return _program_cache[key]


def kernel(x, w_qkv, w_proj, b_proj, block_mask):
    x = np.asarray(x, np.float32)
    w_qkv = np.asarray(w_qkv, np.float32)
    w_proj = np.asarray(w_proj, np.float32)
    b_proj = np.asarray(b_proj, np.float32)
    nc = get_program(block_mask)
    in_maps = _host_prep(x, w_qkv, w_proj)
    res = run_bass_kernel_spmd(nc, in_maps, core_ids=list(range(N_CORES)))
    out = np.empty((B, S, D), np.float32)
    for b in range(B):
        acc = np.asarray(res.results[4 * b]["out"], np.float64)
        for g in range(1, 4):
            acc = acc + np.asarray(res.results[4 * b + g]["out"], np.float64)
        out[b] = (acc + b_proj).astype(np.float32)
    return out


# revision 41
# speedup vs baseline: 1.3268x; 1.0032x over previous
"""Block-sparse multi-head attention on 8 Trainium2 NeuronCores.

Problem: y = proj(softmax(mask(q @ k^T / sqrt(hd))) @ v) for
B=2, S=2048, D=1024, H=16 heads, block size 128, with a [16,16] boolean
block mask (True = masked) applied to strictly-upper (k-block > q-block)
blocks.

Sharding: batch x head-group. Core c handles batch c//4 and heads
[4*(c%4), 4*(c%4)+4). No collectives: the host pre-slices inputs
(including pre-transposing x to x^T) and sums the 4 per-batch partial
projection outputs on the way out.

This version fuses all phases into one software-pipelined instruction
stream to keep ScalarE (the exp bottleneck, ~100us/core) and the PE
(~123us/core) simultaneously busy:
  - x/w_qkv/w_v are uploaded in bf16 (halves input DMA to ~6.5MB);
    DMAs are chunked by xT column-slice and issued in consumption order
    so the first attention exp lands ~8us into the kernel.
  - qk-gen for head pair 0 runs first; v-gen and pair-1 qk-gen chunks
    are interleaved into the attention pipeline of heads 1 and 0
    (sharing one PSUM ring) so the PE never idles long enough for HAM
    to re-throttle the clock.
  - attention per head runs as two window passes g=0/1 (pa [65,1024]
    PSUM x2-ring), per k-block: S^T = kpad_ik @ q^T (runs), P~^T =
    exp(S^T/8) (ScalarE, one op per (ik, 1024-window)), PV accumulated
    into pa with the ones-column denominator trick (row 64).
  - normalization: only the two PSUM->SBUF copies are eager; the
    reciprocal/broadcast/multiply chain (which round-trips SBUF DMAs)
    is deferred and spread over the next head's iterations so it never
    head-of-line-blocks the in-order Vector/GpSimd queues.
  - projection is a 4-deep PSUM pipeline (alternating ring slots) with
    PSUM->SBUF copies alternating Vector/Scalar and per-tile output
    DMAs; m-tiles ordered so the last head's deferred normalize chain
    overlaps the first half of proj.
"""

import numpy as np
from ml_dtypes import bfloat16

import concourse.mybir as mybir
import concourse.tile as tile
from concourse import bacc
from concourse.bass_utils import run_bass_kernel_spmd

B, S, D, H = 2, 2048, 1024, 16
HD = 64          # head dim
BS = 128         # mask block size
NB = S // BS     # 16 blocks per axis
HPC = 4          # heads per core
N_CORES = 8
SCALE = HD ** -0.5
KT = D // 128    # 8 k-tiles over the embedding dim
VW = HPC * (HD + 1)  # 260

F32 = mybir.dt.float32
F32R = mybir.dt.float32r
BF16 = mybir.dt.bfloat16
EXP = mybir.ActivationFunctionType.Exp

_program_cache: dict[bytes, object] = {}


def _plan_runs_g(vis, last_vis, ik, g):
    """Contiguous visible q-block runs for k-block ik within 1024-col
    window g. Runs break at 4-block (512-col = PSUM bank) boundaries."""
    runs = []
    jq, end = 8 * g, 8 * g + 8
    while jq < end:
        if not vis[jq][ik]:
            jq += 1
            continue
        start = jq
        while jq + 1 < end and vis[jq + 1][ik] and (jq + 1) % 4 != 0:
            jq += 1
        stopf = any(last_vis[b] == ik for b in range(start, jq + 1))
        runs.append((start, jq - start + 1, stopf))
        jq += 1
    return runs


def _build_program(mask: np.ndarray):
    vis = [[ik <= jq or not bool(mask[jq, ik]) for ik in range(NB)]
           for jq in range(NB)]
    last_vis = [max(ik for ik in range(NB) if vis[jq][ik]) for jq in range(NB)]
    lastw = [max(last_vis[w * 4:(w + 1) * 4]) for w in range(4)]
    RUNS = {(g, ik): _plan_runs_g(vis, last_vis, ik, g)
            for g in range(2) for ik in range(NB)}

    nc = bacc.Bacc("TRN2", target_bir_lowering=False, debug=False,
                   num_devices=N_CORES)
    # host pre-packs everything 128-partition-major and fully contiguous:
    # xT_sl: [128, 4*8*512]  slice-major: slice s (512 seq cols), then k-tile
    # wqk_sl: [128, 8*512]   k-tile major; within: [q0|q1][k0|k1][q2|q3][k2|k3]
    # wv_sl:  [128, 8*260]   k-tile major
    xT_d = nc.dram_tensor("xT", [128, 4 * KT * 512], BF16, kind="ExternalInput")
    wqk_d = nc.dram_tensor("wqk", [128, KT * 512], BF16, kind="ExternalInput")
    wv_d = nc.dram_tensor("wv", [128, KT * VW], BF16, kind="ExternalInput")
    wpr_d = nc.dram_tensor("wpr", [HPC * HD, D], F32R, kind="ExternalInput")
    out_d = nc.dram_tensor("out", [S, D], BF16, kind="ExternalOutput")

    with tile.TileContext(nc) as tc:
        with tc.tile_pool(name="pp", bufs=1) as pp, \
             tc.tile_pool(name="ptp", bufs=5) as ptp, \
             tc.tile_pool(name="ps", bufs=2, space="PSUM") as ps:
            # ---- persistent SBUF tiles ----
            xT_sl = pp.tile([128, 4 * KT * 512], BF16, tag="xT", name="xT")
            wqk_sl = pp.tile([128, KT * 512], BF16, tag="wqk", name="wqk")
            wv_sl = pp.tile([128, KT * VW], BF16, tag="wv", name="wv")
            wpr_t = [pp.tile([128, D], F32R, tag=f"wpr{k}", name=f"wpr{k}")
                     for k in range(2)]
            q_t = [pp.tile([128, S], F32R, tag=f"q{p}", name=f"q{p}")
                   for p in range(2)]
            kpad_t = [pp.tile([128, S], F32R, tag=f"kp{h}", name=f"kp{h}")
                      for h in range(HPC)]
            v_t = [pp.tile([128, VW], F32R, tag=f"v{m}", name=f"v{m}")
                   for m in range(NB)]
            attn_t = [pp.tile([128, S], F32R, tag=f"attn{i}", name=f"attn{i}")
                      for i in range(2)]
            d16_t = pp.tile([128, 8 * HPC * 2], F32, tag="d16", name="d16")
            r0_t = pp.tile([1, S], F32, tag="r0", name="r0")
            onec = pp.tile([128, 4], F32, tag="onec", name="onec")
            zsrc = pp.tile([64, 512], F32, tag="zsrc", name="zsrc")
            scr = pp.tile([128, 4], F32, tag="scr", name="scr")

            # ---- init + ACT table pre-warm ----
            nc.vector.memset(onec[:], 1.0)
            nc.vector.memset(zsrc[:], 0.0)
            nc.scalar.activation(scr[:], onec[:], EXP, scale=1.0)
            for h in range(HPC):
                z0 = 64 if h % 2 == 0 else 0
                for c in range(4):
                    eng = nc.vector if (h * 4 + c) % 2 == 0 else nc.scalar
                    cs = c * 512
                    if eng is nc.vector:
                        eng.tensor_copy(kpad_t[h][z0:z0 + 64, cs:cs + 512],
                                        zsrc[:])
                    else:
                        eng.copy(kpad_t[h][z0:z0 + 64, cs:cs + 512], zsrc[:])

            # ---- input DMAs: few big contiguous pieces, consumption order,
            # issue alternating between the sync and gpsimd queues (descriptor
            # generation is ~0.6us each and serial per queue) ----
            dmact = [0]

            def in_dma(dst, src):
                eng = nc.sync if dmact[0] % 2 == 0 else nc.gpsimd
                dmact[0] += 1
                eng.dma_start(out=dst, in_=src)

            for h in range(2):  # wqk: 2 x 512KB
                o = h * 2048
                in_dma(wqk_sl[:, o:o + 2048], wqk_d[:, o:o + 2048])
            for s in (0, 1):    # xT s0, s1: 2 x 512KB each
                for h in range(2):
                    o = s * 4096 + h * 2048
                    in_dma(xT_sl[:, o:o + 2048], xT_d[:, o:o + 2048])
            in_dma(wv_sl[:], wv_d[:])  # 530KB
            for p4 in range(4):  # xT s2+s3: 4 x 512KB
                o = 2 * 4096 + p4 * 2048
                in_dma(xT_sl[:, o:o + 2048], xT_d[:, o:o + 2048])
            for k in range(2):
                in_dma(wpr_t[k][:], wpr_d[k * 128:(k + 1) * 128, :])

            # ---- gen chunk emitters (copies alternate Vector/Scalar) ----
            genct = [0]

            def qk_chunk(p, t, c, lead=False):
                """[128,512] chunk of q-pair (t=0) or k-pair (t=1) tile."""
                pb = ps.tile([128, 512], F32, tag="st", bufs=3, name=f"pb{p}{t}{c}")
                off = p * 256 + t * 128
                cs = c * 512
                for k in range(KT):
                    nc.tensor.matmul(
                        pb[:], wqk_sl[:, k * 512 + off:k * 512 + off + 128],
                        xT_sl[:, c * 4096 + k * 512:c * 4096 + (k + 1) * 512],
                        start=(k == 0), stop=(k == KT - 1))
                genct[0] += 1
                use_sc = genct[0] % 2 == 1
                # keep both half-copies of one chunk on ONE engine: the
                # framework serializes sibling readers cross-engine, which
                # couples the exp stream to the Vector queue otherwise
                if t == 0:
                    if use_sc:
                        nc.scalar.copy(q_t[p][:, cs:cs + 512], pb[:])
                    else:
                        nc.vector.tensor_copy(q_t[p][:, cs:cs + 512], pb[:])
                else:
                    h0, h1 = 2 * p, 2 * p + 1
                    if use_sc:
                        nc.scalar.copy(kpad_t[h0][0:64, cs:cs + 512],
                                       pb[0:64, :])
                        nc.scalar.copy(kpad_t[h1][64:128, cs:cs + 512],
                                       pb[64:128, :])
                    else:
                        nc.vector.tensor_copy(kpad_t[h0][0:64, cs:cs + 512],
                                              pb[0:64, :])
                        nc.vector.tensor_copy(kpad_t[h1][64:128, cs:cs + 512],
                                              pb[64:128, :])

            def v_chunk(m):
                pc = ps.tile([128, 512], F32, tag="st", bufs=3, name=f"pc{m}")
                s, r = m // 4, m % 4
                for k in range(KT):
                    nc.tensor.matmul(
                        pc[:, 0:VW],
                        xT_sl[:, s * 4096 + k * 512 + r * 128:
                               s * 4096 + k * 512 + (r + 1) * 128],
                        wv_sl[:, k * VW:(k + 1) * VW],
                        start=(k == 0), stop=(k == KT - 1))
                nc.vector.tensor_copy(v_t[m][:], pc[:, 0:VW])
                nc.vector.tensor_copy(v_t[m][:, HD::HD + 1], onec[:])

            # ---- deferred-op machinery ----
            deferred = []  # [countdown, fn]

            def poll_deferred():
                due = [d for d in deferred if d[0] <= 1]
                for d in due:
                    deferred.remove(d)
                for d in deferred:
                    d[0] -= 1
                for d in due:
                    d[1]()

            def force_deferred(keep=None):
                kept = []
                while deferred:
                    d = deferred.pop(0)
                    if keep is not None and d[2] == keep:
                        kept.append(d)
                    else:
                        d[1]()
                deferred.extend(kept)

            # ---- normalize chain ----
            first_mm = {}   # (j, w) -> True once consumed
            wins_done = {}  # (j, g) -> count

            # per-head staging, ring-allocated (lifetimes span into next head).
            # stage[0:64] = unnormalized attn rows, stage[64:65] = denominator.
            cur = {"stage": None, "odd": None}

            def enqueue_chain(j, g, spacing):
                p, gc = j // 2, g * 1024
                sl = d16_t[:, (2 * j + g) * 8:(2 * j + g + 1) * 8]
                stage, odd = cur["stage"], cur["odd"]
                if j % 2 == 0:
                    dst = attn_t[p][0:64, gc:gc + 1024]
                else:
                    dst = odd[0:64, gc:gc + 1024]

                def s1():
                    nc.gpsimd.dma_start(out=sl, in_=stage[64:65, gc:gc + 1024])

                def s2():
                    nc.vector.reciprocal(sl, sl)

                def s3():
                    nc.gpsimd.dma_start(out=r0_t[0:1, gc:gc + 1024], in_=sl)

                def s4(h):
                    hc = gc + h * 512
                    dbc = pp.tile([64, 512], F32, tag="dbc", bufs=4,
                                  name=f"dbc{j}{g}{h}")
                    cur[f"dbc{j}{g}{h}"] = dbc
                    nc.gpsimd.partition_broadcast(dbc[:],
                                                  r0_t[0:1, hc:hc + 512])

                def s5(h):
                    hc = gc + h * 512
                    dbc = cur.pop(f"dbc{j}{g}{h}")
                    nc.vector.tensor_mul(dst[:, h * 512:(h + 1) * 512],
                                         stage[0:64, hc:hc + 512], dbc[:])

                def s6():
                    nc.gpsimd.dma_start(out=attn_t[p][64:128, gc:gc + 1024],
                                        in_=odd[0:64, gc:gc + 1024])

                # both PBs issued before the MULs so the GpSimd latency is
                # hidden before the Vector ops need the result
                steps = [(1, s1), (2, s2), (1, s3),
                         (1, lambda: s4(0)), (1, lambda: s4(1)),
                         (2, lambda: s5(0)), (1, lambda: s5(1))]
                if j % 2 == 1:
                    steps.append((1, s6))
                cd = 0
                for extra, fn in steps:
                    cd += spacing * extra
                    deferred.append([cd, fn, (j, g)])

            def norm_copies(j, g, ik, pa_g):
                for w in (2 * g, 2 * g + 1):
                    if lastw[w] != ik:
                        continue
                    ws = w * 512
                    rel = ws - g * 1024
                    nc.vector.tensor_copy(cur["stage"][0:65, ws:ws + 512],
                                          pa_g[0:65, rel:rel + 512])
                    wins_done[(j, g)] = wins_done.get((j, g), 0) + 1
                    if wins_done[(j, g)] == 2:
                        enqueue_chain(j, g, spacing=(1 if j == 2 else 2))

            # ---- attention pipeline ----
            pending = [None]  # [(j, g, ik, runs, ptg, pa_g)]

            def flush_pending():
                item = pending[0]
                pending[0] = None
                if item is None:
                    return
                j, g, ik, runs, ptg, pa_g = item
                lhsT_v = v_t[ik][:, j * (HD + 1):(j + 1) * (HD + 1)]
                for (qb0, nbk, stopf) in runs:
                    qs, qlen = qb0 * 128, nbk * 128
                    rel = qs - g * 1024
                    w = qb0 // 4
                    startf = first_mm.pop((j, w), False)
                    nc.tensor.matmul(pa_g[0:65, rel:rel + qlen], lhsT_v,
                                     ptg[:, rel:rel + qlen],
                                     start=startf, stop=stopf,
                                     skip_group_check=True)
                norm_copies(j, g, ik, pa_g)

            def attn_iter(j, g, ik, pa_g, gen=None):
                runs = RUNS[(g, ik)]
                stg = ps.tile([128, 1024], F32, tag="st", bufs=3, name=f"st{j}{g}{ik}")
                lhsT_k = kpad_t[j][:, ik * 128:(ik + 1) * 128]
                qtile = q_t[j // 2]
                for (qb0, nbk, stopf) in runs:
                    qs, qlen = qb0 * 128, nbk * 128
                    rel = qs - g * 1024
                    nc.tensor.matmul(stg[:, rel:rel + qlen], lhsT_k,
                                     qtile[:, qs:qs + qlen],
                                     start=True, stop=True)
                lo = min(r[0] for r in runs) * 128 - g * 1024
                hi = (max(r[0] + r[1] for r in runs)) * 128 - g * 1024
                ptg = ptp.tile([128, 1024], F32R, tag="pt", name=f"pt{j}{g}{ik}")
                nc.scalar.activation(ptg[:, lo:hi], stg[:, lo:hi], EXP,
                                     scale=SCALE)
                if gen is not None:
                    gen()
                poll_deferred()
                flush_pending()
                pending[0] = (j, g, ik, runs, ptg, pa_g)

            # ---- lead: qk-gen for pair 0, windows g=0 ----
            for (t, c) in ((0, 0), (1, 0), (0, 1), (1, 1)):
                qk_chunk(0, t, c, lead=True)

            iters = {g: [ik for ik in range(NB) if RUNS[(g, ik)]]
                     for g in range(2)}

            def head_items(j):
                # g=0 fully then g=1: only ONE pa tile live at a time, which
                # frees 2 PSUM banks for a 3-deep stg ring (the exp pipeline)
                phases = [(0, iters[0]), (1, iters[1])]
                return [(g, ik) for (g, iklist) in phases for ik in iklist]

            def edf_schedule(items, chunks):
                """Assign gen chunks to iteration slots by earliest deadline.
                chunks: list of (deadline_slot_inclusive, fn). Returns
                slot -> [fns]; infeasible chunks go to slot 0."""
                slots = {i: [] for i in range(len(items))}
                fill = {i: 0 for i in range(len(items))}
                for dl, fn in sorted(chunks, key=lambda c: c[0]):
                    placed = False
                    # latest-fit: emit just-in-time so gen MMs queue behind
                    # already-arrived DMA data instead of stalling the PE
                    for s in range(min(dl, len(items) - 1), -1, -1):
                        if fill[s] < 2:
                            slots[s].append(fn)
                            fill[s] += 1
                            placed = True
                            break
                    if not placed:
                        slots[0].insert(0, fn)
                return slots

            def head1_chunks(items):
                """v tiles (PV deadline) + pair-0 windows g=1 (QK deadline)."""
                chunks = []
                for m in range(NB):
                    idx = min((i for i, (g, ik) in enumerate(items) if ik == m),
                              default=0)
                    chunks.append((idx + 1, lambda m=m: v_chunk(m)))
                for c in (2, 3):
                    # q chunk c: first QK of window-pair g=c//2 touching it
                    idx = min((i for i, (g, ik) in enumerate(items)
                               if g == c // 2), default=1)
                    chunks.append((max(0, idx - 1),
                                   lambda c=c: qk_chunk(0, 0, c)))
                    # kpad chunk c: first QK with ik in [4c, 4c+4)
                    idx = min((i for i, (g, ik) in enumerate(items)
                               if 4 * c <= ik < 4 * c + 4), default=1)
                    chunks.append((max(0, idx - 1),
                                   lambda c=c: qk_chunk(0, 1, c)))
                return chunks

            def head0_chunks(items):
                """pair-1 gen, needed only by heads 3/2: spread evenly."""
                chunks = []
                pos = 0
                for c in range(4):
                    for t in (0, 1):
                        chunks.append((pos, lambda t=t, c=c: qk_chunk(1, t, c)))
                        pos += 3
                return chunks

            for j in (1, 0, 3, 2):
                items = head_items(j)
                if j == 1:
                    genmap = edf_schedule(items, head1_chunks(items))
                elif j == 0:
                    genmap = edf_schedule(items, head0_chunks(items))
                else:
                    genmap = {}
                for w in range(4):
                    first_mm[(j, w)] = True
                cur["stage"] = pp.tile([65, S], F32, tag="stage", bufs=2,
                                       name=f"stage{j}")
                if j % 2 == 1:
                    cur["odd"] = pp.tile([64, S], F32R, tag="odd", bufs=1,
                                         name=f"odd{j}")
                pa = {}
                for i, (g, ik) in enumerate(items):
                    if g not in pa:
                        pa[g] = ps.tile([65, 1024], F32, tag="pa", bufs=1,
                                        name=f"pa{j}{g}")
                    fns = genmap.get(i, [])
                    gen = (lambda fns=fns: [f() for f in fns]) if fns else None
                    attn_iter(j, g, ik, pa[g], gen=gen)
                flush_pending()
                for w in range(4):
                    first_mm.pop((j, w), None)

            # ---- projection + output ----
            # flush all chains except the last head's g=1 (interleaved below)
            force_deferred(keep=(2, 1))
            last_chain = [d for d in deferred if d[2] == (2, 1)]
            deferred.clear()
            for mi, m in enumerate(list(range(8)) + list(range(8, 16))):
                if last_chain:
                    last_chain.pop(0)[1]()
                if m == 8:
                    while last_chain:
                        last_chain.pop(0)[1]()
                po = ps.tile([128, D], F32, tag=("st" if mi % 2 == 0 else "pa"),
                             bufs=(3 if mi % 2 == 0 else 1), name=f"po{m}")
                for kt in range(2):
                    for c in range(2):
                        nc.tensor.matmul(
                            po[:, c * 512:(c + 1) * 512],
                            attn_t[kt][:, m * 128:(m + 1) * 128],
                            wpr_t[kt][:, c * 512:(c + 1) * 512],
                            start=(kt == 0), stop=(kt == 1))
                ob = pp.tile([128, D], BF16, tag="ob", bufs=3, name=f"ob{m}")
                if mi % 2 == 0:
                    nc.vector.tensor_copy(ob[:], po[:])
                    nc.sync.dma_start(out=out_d[m * 128:(m + 1) * 128, :],
                                      in_=ob[:])
                else:
                    nc.scalar.copy(ob[:], po[:])
                    nc.gpsimd.dma_start(out=out_d[m * 128:(m + 1) * 128, :],
                                        in_=ob[:])
            while last_chain:
                last_chain.pop(0)[1]()

    # consume first_mm flags at first-visible ik
    nc.compile()
    return nc


def _host_prep(x, w_qkv, w_proj):
    """Per-core input slices, packed 128-partition-major and contiguous.
    x/wqk/wv in bf16, wpr in f32."""
    # xT_sl[b]: [128, 4*8*512] slice-major then k-tile-major
    xT_sl = []
    for b in range(B):
        xT = x[b].T.astype(bfloat16)  # [D, S]
        arr = np.empty((128, 4 * KT * 512), bfloat16)
        for s in range(4):
            for k in range(KT):
                arr[:, s * 4096 + k * 512:s * 4096 + (k + 1) * 512] = \
                    xT[k * 128:(k + 1) * 128, s * 512:(s + 1) * 512]
        xT_sl.append(np.ascontiguousarray(arr))
    in_maps = []
    for c in range(N_CORES):
        b, grp = c // 4, c % 4
        heads = list(range(grp * HPC, (grp + 1) * HPC))
        wqk = np.empty((D, 2 * HPC * HD), np.float32)
        wv = np.zeros((D, VW), np.float32)
        wpr = np.empty((HPC * HD, D), np.float32)
        for j, h in enumerate(heads):
            p, i = j // 2, j % 2  # pair, index in pair
            # pair block: [q_a|q_b][k_a|k_b] at 256*p
            wqk[:, p * 256 + i * HD:p * 256 + (i + 1) * HD] = \
                w_qkv[:, h * HD:(h + 1) * HD]
            wqk[:, p * 256 + 128 + i * HD:p * 256 + 128 + (i + 1) * HD] = \
                w_qkv[:, D + h * HD:D + (h + 1) * HD]
            wv[:, j * (HD + 1):j * (HD + 1) + HD] = \
                w_qkv[:, 2 * D + h * HD:2 * D + (h + 1) * HD]
            wpr[j * HD:(j + 1) * HD, :] = w_proj[h * HD:(h + 1) * HD, :]
        # repack k-tile-major [128, KT*cols]
        wqk_sl = np.empty((128, KT * 512), bfloat16)
        wv_sl = np.empty((128, KT * VW), bfloat16)
        for k in range(KT):
            wqk_sl[:, k * 512:(k + 1) * 512] = \
                wqk[k * 128:(k + 1) * 128, :].astype(bfloat16)
            wv_sl[:, k * VW:(k + 1) * VW] = \
                wv[k * 128:(k + 1) * 128, :].astype(bfloat16)
        in_maps.append({
            "xT": xT_sl[b],
            "wqk": np.ascontiguousarray(wqk_sl),
            "wv": np.ascontiguousarray(wv_sl),
            "wpr": np.ascontiguousarray(wpr),
        })
    return in_maps


def get_program(block_mask: np.ndarray):
    key = np.asarray(block_mask, bool).tobytes()
    if key not in _program_cache:
        _program_cache[key] = _build_program(np.asarray(block_mask, bool))
    return _program_cache[key]


def kernel(x, w_qkv, w_proj, b_proj, block_mask):
    x = np.asarray(x, np.float32)
    w_qkv = np.asarray(w_qkv, np.float32)
    w_proj = np.asarray(w_proj, np.float32)
    b_proj = np.asarray(b_proj, np.float32)
    nc = get_program(block_mask)
    in_maps = _host_prep(x, w_qkv, w_proj)
    res = run_bass_kernel_spmd(nc, in_maps, core_ids=list(range(N_CORES)))
    out = np.empty((B, S, D), np.float32)
    for b in range(B):
        acc = np.asarray(res.results[4 * b]["out"], np.float64)
        for g in range(1, 4):
            acc = acc + np.asarray(res.results[4 * b + g]["out"], np.float64)
        out[b] = (acc + b_proj).astype(np.float32)
    return out


# revision 43
# speedup vs baseline: 1.3310x; 1.0031x over previous
"""Block-sparse multi-head attention on 8 Trainium2 NeuronCores.

Problem: y = proj(softmax(mask(q @ k^T / sqrt(hd))) @ v) for
B=2, S=2048, D=1024, H=16 heads, block size 128, with a [16,16] boolean
block mask (True = masked) applied to strictly-upper (k-block > q-block)
blocks.

Sharding: batch x head-group. Core c handles batch c//4 and heads
[4*(c%4), 4*(c%4)+4). No collectives: the host pre-slices inputs
(including pre-transposing x to x^T) and sums the 4 per-batch partial
projection outputs on the way out.

This version fuses all phases into one software-pipelined instruction
stream to keep ScalarE (the exp bottleneck, ~100us/core) and the PE
(~123us/core) simultaneously busy:
  - x/w_qkv/w_v are uploaded in bf16 (halves input DMA to ~6.5MB);
    DMAs are chunked by xT column-slice and issued in consumption order
    so the first attention exp lands ~8us into the kernel.
  - qk-gen for head pair 0 runs first; v-gen and pair-1 qk-gen chunks
    are interleaved into the attention pipeline of heads 1 and 0
    (sharing one PSUM ring) so the PE never idles long enough for HAM
    to re-throttle the clock.
  - attention per head runs as two window passes g=0/1 (pa [65,1024]
    PSUM x2-ring), per k-block: S^T = kpad_ik @ q^T (runs), P~^T =
    exp(S^T/8) (ScalarE, one op per (ik, 1024-window)), PV accumulated
    into pa with the ones-column denominator trick (row 64).
  - normalization: only the two PSUM->SBUF copies are eager; the
    reciprocal/broadcast/multiply chain (which round-trips SBUF DMAs)
    is deferred and spread over the next head's iterations so it never
    head-of-line-blocks the in-order Vector/GpSimd queues.
  - projection is a 4-deep PSUM pipeline (alternating ring slots) with
    PSUM->SBUF copies alternating Vector/Scalar and per-tile output
    DMAs; m-tiles ordered so the last head's deferred normalize chain
    overlaps the first half of proj.
"""

import numpy as np
from ml_dtypes import bfloat16

import concourse.mybir as mybir
import concourse.tile as tile
from concourse import bacc
from concourse.bass_utils import run_bass_kernel_spmd

B, S, D, H = 2, 2048, 1024, 16
HD = 64          # head dim
BS = 128         # mask block size
NB = S // BS     # 16 blocks per axis
HPC = 4          # heads per core
N_CORES = 8
SCALE = HD ** -0.5
KT = D // 128    # 8 k-tiles over the embedding dim
VW = HPC * (HD + 1)  # 260

F32 = mybir.dt.float32
F32R = mybir.dt.float32r
BF16 = mybir.dt.bfloat16
EXP = mybir.ActivationFunctionType.Exp

_program_cache: dict[bytes, object] = {}


def _plan_runs_g(vis, last_vis, ik, g):
    """Contiguous visible q-block runs for k-block ik within 1024-col
    window g. Runs break at 4-block (512-col = PSUM bank) boundaries."""
    runs = []
    jq, end = 8 * g, 8 * g + 8
    while jq < end:
        if not vis[jq][ik]:
            jq += 1
            continue
        start = jq
        while jq + 1 < end and vis[jq + 1][ik] and (jq + 1) % 4 != 0:
            jq += 1
        stopf = any(last_vis[b] == ik for b in range(start, jq + 1))
        runs.append((start, jq - start + 1, stopf))
        jq += 1
    return runs


def _build_program(mask: np.ndarray):
    vis = [[ik <= jq or not bool(mask[jq, ik]) for ik in range(NB)]
           for jq in range(NB)]
    last_vis = [max(ik for ik in range(NB) if vis[jq][ik]) for jq in range(NB)]
    lastw = [max(last_vis[w * 4:(w + 1) * 4]) for w in range(4)]
    RUNS = {(g, ik): _plan_runs_g(vis, last_vis, ik, g)
            for g in range(2) for ik in range(NB)}

    nc = bacc.Bacc("TRN2", target_bir_lowering=False, debug=False,
                   num_devices=N_CORES)
    # host pre-packs everything 128-partition-major and fully contiguous:
    # xT_sl: [128, 4*8*512]  slice-major: slice s (512 seq cols), then k-tile
    # wqk_sl: [128, 8*512]   k-tile major; within: [q0|q1][k0|k1][q2|q3][k2|k3]
    # wv_sl:  [128, 8*260]   k-tile major
    xT_d = nc.dram_tensor("xT", [128, 4 * KT * 512], BF16, kind="ExternalInput")
    wqk_d = nc.dram_tensor("wqk", [128, KT * 512], BF16, kind="ExternalInput")
    wv_d = nc.dram_tensor("wv", [128, KT * VW], BF16, kind="ExternalInput")
    wpr_d = nc.dram_tensor("wpr", [HPC * HD, D], F32R, kind="ExternalInput")
    out_d = nc.dram_tensor("out", [S, D], BF16, kind="ExternalOutput")

    with tile.TileContext(nc) as tc:
        with tc.tile_pool(name="pp", bufs=1) as pp, \
             tc.tile_pool(name="ptp", bufs=5) as ptp, \
             tc.tile_pool(name="ps", bufs=2, space="PSUM") as ps:
            # ---- persistent SBUF tiles ----
            xT_sl = pp.tile([128, 4 * KT * 512], BF16, tag="xT", name="xT")
            wqk_sl = pp.tile([128, KT * 512], BF16, tag="wqk", name="wqk")
            wv_sl = pp.tile([128, KT * VW], BF16, tag="wv", name="wv")
            wpr_t = [pp.tile([128, D], F32R, tag=f"wpr{k}", name=f"wpr{k}")
                     for k in range(2)]
            q_t = [pp.tile([128, S], F32R, tag=f"q{p}", name=f"q{p}")
                   for p in range(2)]
            kpad_t = [pp.tile([128, S], F32R, tag=f"kp{h}", name=f"kp{h}")
                      for h in range(HPC)]
            v_t = [pp.tile([128, VW], F32R, tag=f"v{m}", name=f"v{m}")
                   for m in range(NB)]
            attn_t = [pp.tile([128, S], F32R, tag=f"attn{i}", name=f"attn{i}")
                      for i in range(2)]
            d16_t = pp.tile([128, 8 * HPC * 2], F32, tag="d16", name="d16")
            r0_t = pp.tile([1, S], F32, tag="r0", name="r0")
            onec = pp.tile([128, 4], F32, tag="onec", name="onec")
            zsrc = pp.tile([64, 512], F32, tag="zsrc", name="zsrc")
            scr = pp.tile([128, 4], F32, tag="scr", name="scr")

            # ---- init + ACT table pre-warm ----
            nc.vector.memset(onec[:], 1.0)
            nc.vector.memset(zsrc[:], 0.0)
            nc.scalar.activation(scr[:], onec[:], EXP, scale=1.0)
            for h in range(HPC):
                z0 = 64 if h % 2 == 0 else 0
                for c in range(4):
                    eng = nc.vector if (h * 4 + c) % 2 == 0 else nc.scalar
                    cs = c * 512
                    if eng is nc.vector:
                        eng.tensor_copy(kpad_t[h][z0:z0 + 64, cs:cs + 512],
                                        zsrc[:])
                    else:
                        eng.copy(kpad_t[h][z0:z0 + 64, cs:cs + 512], zsrc[:])

            # ---- input DMAs: few big contiguous pieces, consumption order,
            # issue alternating between the sync and gpsimd queues (descriptor
            # generation is ~0.6us each and serial per queue) ----
            dmact = [0]

            def in_dma(dst, src):
                eng = nc.sync if dmact[0] % 2 == 0 else nc.gpsimd
                dmact[0] += 1
                eng.dma_start(out=dst, in_=src)

            for h in range(4):  # wqk: 4 x 256KB
                o = h * 1024
                in_dma(wqk_sl[:, o:o + 1024], wqk_d[:, o:o + 1024])
            for h in range(4):  # xT s0: 4 x 256KB
                o = h * 1024
                in_dma(xT_sl[:, o:o + 1024], xT_d[:, o:o + 1024])
            for h in range(2):  # xT s1: 2 x 512KB
                o = 4096 + h * 2048
                in_dma(xT_sl[:, o:o + 2048], xT_d[:, o:o + 2048])
            in_dma(wv_sl[:], wv_d[:])  # 530KB
            for p4 in range(4):  # xT s2+s3: 4 x 512KB
                o = 2 * 4096 + p4 * 2048
                in_dma(xT_sl[:, o:o + 2048], xT_d[:, o:o + 2048])
            for k in range(2):
                in_dma(wpr_t[k][:], wpr_d[k * 128:(k + 1) * 128, :])

            # ---- gen chunk emitters (copies alternate Vector/Scalar) ----
            genct = [0]

            def qk_chunk(p, t, c, lead=False):
                """[128,512] chunk of q-pair (t=0) or k-pair (t=1) tile."""
                pb = ps.tile([128, 512], F32, tag="st", bufs=3, name=f"pb{p}{t}{c}")
                off = p * 256 + t * 128
                cs = c * 512
                for k in range(KT):
                    nc.tensor.matmul(
                        pb[:], wqk_sl[:, k * 512 + off:k * 512 + off + 128],
                        xT_sl[:, c * 4096 + k * 512:c * 4096 + (k + 1) * 512],
                        start=(k == 0), stop=(k == KT - 1))
                genct[0] += 1
                use_sc = genct[0] % 2 == 1
                # keep both half-copies of one chunk on ONE engine: the
                # framework serializes sibling readers cross-engine, which
                # couples the exp stream to the Vector queue otherwise
                if t == 0:
                    if use_sc:
                        nc.scalar.copy(q_t[p][:, cs:cs + 512], pb[:])
                    else:
                        nc.vector.tensor_copy(q_t[p][:, cs:cs + 512], pb[:])
                else:
                    h0, h1 = 2 * p, 2 * p + 1
                    if use_sc:
                        nc.scalar.copy(kpad_t[h0][0:64, cs:cs + 512],
                                       pb[0:64, :])
                        nc.scalar.copy(kpad_t[h1][64:128, cs:cs + 512],
                                       pb[64:128, :])
                    else:
                        nc.vector.tensor_copy(kpad_t[h0][0:64, cs:cs + 512],
                                              pb[0:64, :])
                        nc.vector.tensor_copy(kpad_t[h1][64:128, cs:cs + 512],
                                              pb[64:128, :])

            def v_chunk(m):
                pc = ps.tile([128, 512], F32, tag="st", bufs=3, name=f"pc{m}")
                s, r = m // 4, m % 4
                for k in range(KT):
                    nc.tensor.matmul(
                        pc[:, 0:VW],
                        xT_sl[:, s * 4096 + k * 512 + r * 128:
                               s * 4096 + k * 512 + (r + 1) * 128],
                        wv_sl[:, k * VW:(k + 1) * VW],
                        start=(k == 0), stop=(k == KT - 1))
                nc.vector.tensor_copy(v_t[m][:], pc[:, 0:VW])
                nc.vector.tensor_copy(v_t[m][:, HD::HD + 1], onec[:])

            # ---- deferred-op machinery ----
            deferred = []  # [countdown, fn]

            def poll_deferred():
                due = [d for d in deferred if d[0] <= 1]
                for d in due:
                    deferred.remove(d)
                for d in deferred:
                    d[0] -= 1
                for d in due:
                    d[1]()

            def force_deferred(keep=None):
                kept = []
                while deferred:
                    d = deferred.pop(0)
                    if keep is not None and d[2] == keep:
                        kept.append(d)
                    else:
                        d[1]()
                deferred.extend(kept)

            # ---- normalize chain ----
            first_mm = {}   # (j, w) -> True once consumed
            wins_done = {}  # (j, g) -> count

            # per-head staging, ring-allocated (lifetimes span into next head).
            # stage[0:64] = unnormalized attn rows, stage[64:65] = denominator.
            cur = {"stage": None, "odd": None}

            def enqueue_chain(j, g, spacing):
                p, gc = j // 2, g * 1024
                sl = d16_t[:, (2 * j + g) * 8:(2 * j + g + 1) * 8]
                stage, odd = cur["stage"], cur["odd"]
                if j % 2 == 0:
                    dst = attn_t[p][0:64, gc:gc + 1024]
                else:
                    dst = odd[0:64, gc:gc + 1024]

                def s1():
                    nc.gpsimd.dma_start(out=sl, in_=stage[64:65, gc:gc + 1024])

                def s2():
                    nc.vector.reciprocal(sl, sl)

                def s3():
                    nc.gpsimd.dma_start(out=r0_t[0:1, gc:gc + 1024], in_=sl)

                def s4(h):
                    hc = gc + h * 512
                    dbc = pp.tile([64, 512], F32, tag="dbc", bufs=4,
                                  name=f"dbc{j}{g}{h}")
                    cur[f"dbc{j}{g}{h}"] = dbc
                    nc.gpsimd.partition_broadcast(dbc[:],
                                                  r0_t[0:1, hc:hc + 512])

                def s5(h):
                    hc = gc + h * 512
                    dbc = cur.pop(f"dbc{j}{g}{h}")
                    nc.vector.tensor_mul(dst[:, h * 512:(h + 1) * 512],
                                         stage[0:64, hc:hc + 512], dbc[:])

                def s6():
                    nc.gpsimd.dma_start(out=attn_t[p][64:128, gc:gc + 1024],
                                        in_=odd[0:64, gc:gc + 1024])

                # both PBs issued before the MULs so the GpSimd latency is
                # hidden before the Vector ops need the result
                steps = [(1, s1), (2, s2), (1, s3),
                         (1, lambda: s4(0)), (1, lambda: s4(1)),
                         (2, lambda: s5(0)), (1, lambda: s5(1))]
                if j % 2 == 1:
                    steps.append((1, s6))
                cd = 0
                for extra, fn in steps:
                    cd += spacing * extra
                    deferred.append([cd, fn, (j, g)])

            def norm_copies(j, g, ik, pa_g):
                for w in (2 * g, 2 * g + 1):
                    if lastw[w] != ik:
                        continue
                    ws = w * 512
                    rel = ws - g * 1024
                    nc.vector.tensor_copy(cur["stage"][0:65, ws:ws + 512],
                                          pa_g[0:65, rel:rel + 512])
                    wins_done[(j, g)] = wins_done.get((j, g), 0) + 1
                    if wins_done[(j, g)] == 2:
                        enqueue_chain(j, g, spacing=(1 if j == 2 else 2))

            # ---- attention pipeline ----
            pending = [None]  # [(j, g, ik, runs, ptg, pa_g)]

            def flush_pending():
                item = pending[0]
                pending[0] = None
                if item is None:
                    return
                j, g, ik, runs, ptg, pa_g = item
                lhsT_v = v_t[ik][:, j * (HD + 1):(j + 1) * (HD + 1)]
                for (qb0, nbk, stopf) in runs:
                    qs, qlen = qb0 * 128, nbk * 128
                    rel = qs - g * 1024
                    w = qb0 // 4
                    startf = first_mm.pop((j, w), False)
                    nc.tensor.matmul(pa_g[0:65, rel:rel + qlen], lhsT_v,
                                     ptg[:, rel:rel + qlen],
                                     start=startf, stop=stopf,
                                     skip_group_check=True)
                norm_copies(j, g, ik, pa_g)

            def attn_iter(j, g, ik, pa_g, gen=None):
                runs = RUNS[(g, ik)]
                stg = ps.tile([128, 1024], F32, tag="st", bufs=3, name=f"st{j}{g}{ik}")
                lhsT_k = kpad_t[j][:, ik * 128:(ik + 1) * 128]
                qtile = q_t[j // 2]
                for (qb0, nbk, stopf) in runs:
                    qs, qlen = qb0 * 128, nbk * 128
                    rel = qs - g * 1024
                    nc.tensor.matmul(stg[:, rel:rel + qlen], lhsT_k,
                                     qtile[:, qs:qs + qlen],
                                     start=True, stop=True)
                lo = min(r[0] for r in runs) * 128 - g * 1024
                hi = (max(r[0] + r[1] for r in runs)) * 128 - g * 1024
                ptg = ptp.tile([128, 1024], F32R, tag="pt", name=f"pt{j}{g}{ik}")
                nc.scalar.activation(ptg[:, lo:hi], stg[:, lo:hi], EXP,
                                     scale=SCALE)
                if gen is not None:
                    gen()
                poll_deferred()
                flush_pending()
                pending[0] = (j, g, ik, runs, ptg, pa_g)

            # ---- lead: qk-gen for pair 0, windows g=0 ----
            for (t, c) in ((0, 0), (1, 0), (0, 1), (1, 1)):
                qk_chunk(0, t, c, lead=True)

            iters = {g: [ik for ik in range(NB) if RUNS[(g, ik)]]
                     for g in range(2)}

            def head_items(j):
                # g=0 fully then g=1: only ONE pa tile live at a time, which
                # frees 2 PSUM banks for a 3-deep stg ring (the exp pipeline)
                phases = [(0, iters[0]), (1, iters[1])]
                return [(g, ik) for (g, iklist) in phases for ik in iklist]

            def edf_schedule(items, chunks):
                """Assign gen chunks to iteration slots by earliest deadline.
                chunks: list of (deadline_slot_inclusive, fn). Returns
                slot -> [fns]; infeasible chunks go to slot 0."""
                slots = {i: [] for i in range(len(items))}
                fill = {i: 0 for i in range(len(items))}
                for dl, fn in sorted(chunks, key=lambda c: c[0]):
                    placed = False
                    # latest-fit: emit just-in-time so gen MMs queue behind
                    # already-arrived DMA data instead of stalling the PE
                    for s in range(min(dl, len(items) - 1), -1, -1):
                        if fill[s] < 2:
                            slots[s].append(fn)
                            fill[s] += 1
                            placed = True
                            break
                    if not placed:
                        slots[0].insert(0, fn)
                return slots

            def head1_chunks(items):
                """v tiles (PV deadline) + pair-0 windows g=1 (QK deadline)."""
                chunks = []
                for m in range(NB):
                    idx = min((i for i, (g, ik) in enumerate(items) if ik == m),
                              default=0)
                    chunks.append((idx + 1, lambda m=m: v_chunk(m)))
                for c in (2, 3):
                    # q chunk c: first QK of window-pair g=c//2 touching it
                    idx = min((i for i, (g, ik) in enumerate(items)
                               if g == c // 2), default=1)
                    chunks.append((max(0, idx - 1),
                                   lambda c=c: qk_chunk(0, 0, c)))
                    # kpad chunk c: first QK with ik in [4c, 4c+4)
                    idx = min((i for i, (g, ik) in enumerate(items)
                               if 4 * c <= ik < 4 * c + 4), default=1)
                    chunks.append((max(0, idx - 1),
                                   lambda c=c: qk_chunk(0, 1, c)))
                return chunks

            def head0_chunks(items):
                """pair-1 gen, needed only by heads 3/2: spread evenly."""
                chunks = []
                pos = 0
                for c in range(4):
                    for t in (0, 1):
                        chunks.append((pos, lambda t=t, c=c: qk_chunk(1, t, c)))
                        pos += 3
                return chunks

            for j in (1, 0, 3, 2):
                items = head_items(j)
                if j == 1:
                    genmap = edf_schedule(items, head1_chunks(items))
                elif j == 0:
                    genmap = edf_schedule(items, head0_chunks(items))
                else:
                    genmap = {}
                for w in range(4):
                    first_mm[(j, w)] = True
                cur["stage"] = pp.tile([65, S], F32, tag="stage", bufs=2,
                                       name=f"stage{j}")
                if j % 2 == 1:
                    cur["odd"] = pp.tile([64, S], F32R, tag="odd", bufs=1,
                                         name=f"odd{j}")
                pa = {}
                for i, (g, ik) in enumerate(items):
                    if g not in pa:
                        pa[g] = ps.tile([65, 1024], F32, tag="pa", bufs=1,
                                        name=f"pa{j}{g}")
                    fns = genmap.get(i, [])
                    gen = (lambda fns=fns: [f() for f in fns]) if fns else None
                    attn_iter(j, g, ik, pa[g], gen=gen)
                flush_pending()
                for w in range(4):
                    first_mm.pop((j, w), None)

            # ---- projection + output ----
            # flush all chains except the last head's g=1 (interleaved below)
            force_deferred(keep=(2, 1))
            last_chain = [d for d in deferred if d[2] == (2, 1)]
            deferred.clear()
            for mi, m in enumerate(list(range(8)) + list(range(8, 16))):
                if last_chain:
                    last_chain.pop(0)[1]()
                if m == 8:
                    while last_chain:
                        last_chain.pop(0)[1]()
                po = ps.tile([128, D], F32, tag="st", bufs=3, name=f"po{m}")
                for kt in range(2):
                    for c in range(2):
                        nc.tensor.matmul(
                            po[:, c * 512:(c + 1) * 512],
                            attn_t[kt][:, m * 128:(m + 1) * 128],
                            wpr_t[kt][:, c * 512:(c + 1) * 512],
                            start=(kt == 0), stop=(kt == 1))
                ob = pp.tile([128, D], BF16, tag="ob", bufs=3, name=f"ob{m}")
                if mi % 2 == 0:
                    nc.vector.tensor_copy(ob[:], po[:])
                    nc.sync.dma_start(out=out_d[m * 128:(m + 1) * 128, :],
                                      in_=ob[:])
                else:
                    nc.scalar.copy(ob[:], po[:])
                    nc.gpsimd.dma_start(out=out_d[m * 128:(m + 1) * 128, :],
                                        in_=ob[:])
            while last_chain:
                last_chain.pop(0)[1]()

    # consume first_mm flags at first-visible ik
    nc.compile()
    return nc


def _host_prep(x, w_qkv, w_proj):
    """Per-core input slices, packed 128-partition-major and contiguous.
    x/wqk/wv in bf16, wpr in f32."""
    # xT_sl[b]: [128, 4*8*512] slice-major then k-tile-major
    xT_sl = []
    for b in range(B):
        xT = x[b].T.astype(bfloat16)  # [D, S]
        arr = np.empty((128, 4 * KT * 512), bfloat16)
        for s in range(4):
            for k in range(KT):
                arr[:, s * 4096 + k * 512:s * 4096 + (k + 1) * 512] = \
                    xT[k * 128:(k + 1) * 128, s * 512:(s + 1) * 512]
        xT_sl.append(np.ascontiguousarray(arr))
    in_maps = []
    for c in range(N_CORES):
        b, grp = c // 4, c % 4
        heads = list(range(grp * HPC, (grp + 1) * HPC))
        wqk = np.empty((D, 2 * HPC * HD), np.float32)
        wv = np.zeros((D, VW), np.float32)
        wpr = np.empty((HPC * HD, D), np.float32)
        for j, h in enumerate(heads):
            p, i = j // 2, j % 2  # pair, index in pair
            # pair block: [q_a|q_b][k_a|k_b] at 256*p
            wqk[:, p * 256 + i * HD:p * 256 + (i + 1) * HD] = \
                w_qkv[:, h * HD:(h + 1) * HD]
            wqk[:, p * 256 + 128 + i * HD:p * 256 + 128 + (i + 1) * HD] = \
                w_qkv[:, D + h * HD:D + (h + 1) * HD]
            wv[:, j * (HD + 1):j * (HD + 1) + HD] = \
                w_qkv[:, 2 * D + h * HD:2 * D + (h + 1) * HD]
            wpr[j * HD:(j + 1) * HD, :] = w_proj[h * HD:(h + 1) * HD, :]
        # repack k-tile-major [128, KT*cols]
        wqk_sl = np.empty((128, KT * 512), bfloat16)
        wv_sl = np.empty((128, KT * VW), bfloat16)
        for k in range(KT):
            wqk_sl[:, k * 512:(k + 1) * 512] = \
                wqk[k * 128:(k + 1) * 128, :].astype(bfloat16)
            wv_sl[:, k * VW:(k + 1) * VW] = \
                wv[k * 128:(k + 1) * 128, :].astype(bfloat16)
        in_maps.append({
            "xT": xT_sl[b],
            "wqk": np.ascontiguousarray(wqk_sl),
            "wv": np.ascontiguousarray(wv_sl),
            "wpr": np.ascontiguousarray(wpr),
        })
    return in_maps


def get_program(block_mask: np.ndarray):
    key = np.asarray(block_mask, bool).tobytes()
    if key not in _program_cache:
        _program_cache[key] = _build_program(np.asarray(block_mask, bool))
    return _program_cache[key]


def kernel(x, w_qkv, w_proj, b_proj, block_mask):
    x = np.asarray(x, np.float32)
    w_qkv = np.asarray(w_qkv, np.float32)
    w_proj = np.asarray(w_proj, np.float32)
    b_proj = np.asarray(b_proj, np.float32)
    nc = get_program(block_mask)
    in_maps = _host_prep(x, w_qkv, w_proj)
    res = run_bass_kernel_spmd(nc, in_maps, core_ids=list(range(N_CORES)))
    out = np.empty((B, S, D), np.float32)
    for b in range(B):
        acc = np.asarray(res.results[4 * b]["out"], np.float64)
        for g in range(1, 4):
            acc = acc + np.asarray(res.results[4 * b + g]["out"], np.float64)
        out[b] = (acc + b_proj).astype(np.float32)
    return out
